# revision 28
# baseline (speedup 1.0000x reference)
"""Trainium2 Bass kernel for EvalMemoryReader (retrieval_knn).

Distributed plan (8 NeuronCores, memory axis THW sharded -> 1 frame/core):
  A. stage-1 fp32 matmul (layout A: mem rows x queries) -> per-row argmax
     via DVE max8+find-index -> gaussian center (y,x) per memory row.
  B. stage-2 fp32 matmul (layout B: queries x mem rows) with augmented
     channels folding the gaussian + per-row constants into the contraction
     -> selection scores s_hat.  Per-16 segment maxes -> AllToAll (query
     sharded) -> rank-51-of-segmaxes threshold t + column max via 7 rounds
     of max8/match_replace -> AllGather.  Local survivor counts + 8 smallest
     survivors per query (masked max8) -> AllGather -> exact v50/v51 of the
     global top-50 boundary -> tau = midpoint.
  C. fp32 matmul (layout A) recomputes scores minus column max; mask at tau
     (exact top-50) * exp -> bf16 weights; bf16 readout matmul with a ones
     row appended for the softmax normalizer; AllReduce; normalize.

kernel() takes FULL inputs, shards host-side, runs SPMD on cores 0-7.
"""

import math
import os

import ml_dtypes
import numpy as np

import concourse.bass as bass
import concourse.bacc as bacc
import concourse.mybir as mybir
from concourse.tile import TileContext

ND = 8
CK, CV, T, H, W = 64, 512, 8, 32, 56
HW = H * W              # 1792 queries
THW = T * HW            # 14336 memory locations
ML = THW // ND          # 1792 memory rows per core (exactly one frame)
NCH = HW // 128         # 14 chunks of 128 (query or mem rows)
NB = 4                  # 448-wide free-dim chunks per 1792
NW = HW // NB           # 448
SEG = 16
NSEG = ML // SEG        # 112 segments per core
NQ = HW // ND           # 224 queries per core for threshold extraction
GD = 2.0 * 5.6 * 5.6    # 62.72
CG = math.sqrt(2.0 / GD)
BIG = 1.0e30
NEG = -1.0e30
MCV = CV // 128         # 4 output chunks

F32 = mybir.dt.float32
F32R = mybir.dt.float32r
BF16 = mybir.dt.bfloat16
U32 = mybir.dt.uint32
ALU = mybir.AluOpType
ACT = mybir.ActivationFunctionType
AX = mybir.AxisListType


class _Trunc(Exception):
    pass


def _build():
    nc = bacc.Bacc(num_devices=ND)

    msa_d = nc.dram_tensor("msa", [65, ML], F32, kind="ExternalInput")
    msb_d = nc.dram_tensor("msb", [68, ML], F32, kind="ExternalInput")
    q1_d = nc.dram_tensor("q1", [65, HW], F32, kind="ExternalInput")
    q2_d = nc.dram_tensor("q2", [67, HW], F32, kind="ExternalInput")
    q2c_d = nc.dram_tensor("q2c", [68, HW], F32, kind="ExternalInput")
    a8t_d = nc.dram_tensor("a8t", [128, NCH], F32, kind="ExternalInput")
    mvt_d = nc.dram_tensor("mvt", [128, NCH * CV], BF16, kind="ExternalInput")
    out_d = nc.dram_tensor("out", [CV // ND + 1, HW], F32, kind="ExternalOutput")

    iota16_c = nc.inline_tensor(
        np.broadcast_to(np.arange(16, dtype=np.float32), (128, 16)).copy(),
        name="iota16")
    ones_1x128_c = nc.inline_tensor(np.ones((1, 128), np.float32), name="o1x128")
    ones_128x1_c = nc.inline_tensor(
        np.ones((128, 1), np.float32).astype(ml_dtypes.bfloat16), name="o128x1")
    thr56_c = nc.inline_tensor(
        np.broadcast_to(np.arange(1, H, dtype=np.float32) * W, (128, H - 1))
        .copy(), name="thr56")

    # collective bounce buffers
    segmax_l = nc.dram_tensor("segmax_l", [HW, NSEG], F32)
    segmax_x = nc.dram_tensor("segmax_x", [HW, NSEG], F32)
    tstats_l = nc.dram_tensor("tstats_l", [NQ, 1], F32)
    tstats_g = nc.dram_tensor("tstats_g", [HW, 1], F32, addr_space="Shared")
    stats_l = nc.dram_tensor("stats_l", [HW, 9], F32)
    stats_g = nc.dram_tensor("stats_g", [ND, HW, 9], F32, addr_space="Shared")
    scr = [nc.dram_tensor(f"scr{i}", [HW], F32) for i in range(6)]
    # readout rows interleaved in groups of 65 per core: rows 65d..65d+63 are
    # value rows 64d..64d+63, row 65d+64 is a copy of the local norm row, so a
    # single ReduceScatter delivers each core its value slice + global norm.
    ro_l = nc.dram_tensor("ro_l", [(CV // ND + 1) * ND, HW], F32)

    groups = [list(range(ND))]

    from contextlib import ExitStack
    with TileContext(nc) as tc, ExitStack() as es:
        try:
            POOL_E = mybir.EngineType.Pool
            cpool = es.enter_context(tc.tile_pool(name="consts", bufs=1))
            def cload(ap, name):
                return cpool.tile_from(ap, force_copy=True, name=name,
                                       forced_dma_engine=POOL_E)
            msa = cload(msa_d[:], "msa_t")
            msb = cload(msb_d[:], "msb_t")
            q1 = cload(q1_d[:], "q1_t")
            q2 = cload(q2_d[:], "q2_t")
            q2c = cload(q2c_d[:], "q2c_t")
            a8t = cload(a8t_d[:], "a8t_t")
            iota16 = cload(iota16_c[:], "iota16_t")
            ones_cb = cload(ones_128x1_c[:], "ones_cb_t")
            thr56 = cload(thr56_c[:], "thr56_t")

            # prefetch mv (readout operand) at t=0 so the readout never
            # stalls on its DMA; lives in its own pool for the whole kernel
            mvpool = es.enter_context(tc.tile_pool(name="mvpool", bufs=1))
            mvt = mvpool.tile_from(mvt_d[:], force_copy=True,
                                   forced_dma_engine=POOL_E)
            mvt3 = mvt.rearrange("p (k c) -> p k c", c=CV)

            spool = es.enter_context(tc.tile_pool(name="smalls", bufs=1))
            ycg_t = spool.tile([128, NCH], F32)
            xcg_t = spool.tile([128, NCH], F32)
            alp_t = spool.tile([128, NCH], F32)
            t_all = spool.tile([128, NCH], F32)
            cnt_t = spool.tile([128, NCH], F32)
            mins_t = spool.tile([128, NCH, 8], F32)
            tauc_t = spool.tile([128, NCH], F32)

            # ---------------- phase A: argmax per memory row ----------------
            # scatter per-row channels into msb rows 64..66 (partition -> free),
            # bounced through DRAM because SBUF<->SBUF transpose APs don't balance
            def part_to_row(scratch, row_ap, tile_ap):
                nc.sync.dma_start(
                    out=scratch[:].rearrange("(m q) -> q m", q=128), in_=tile_ap)
                nc.sync.dma_start(out=row_ap, in_=scratch[:])

            PH = int(os.environ.get("KPHASE", "99"))

            # -------- phases A+B interleaved in one PSUM pool --------
            # A (argmax per memory row) runs single-buffered in 4 banks;
            # B column-blocks (448 memory columns each) run in 2 alternating
            # 1-bank tiles as soon as the gaussian rows they need exist, so
            # B's matmuls/copies/segmax hide entirely under A's DVE spine.
            with tc.tile_pool(name="sBpool", bufs=1) as sBpool:
                sB = sBpool.tile([128, NCH, ML], F32)
                seg_all = sBpool.tile([128, NCH, NSEG], F32)

                def a_chunk(psAB, wkA, m):
                    ps = psAB.tile([128, NB, 512], F32, tag="a")
                    for j in range(NB):
                        nc.tensor.matmul(
                            ps[:, j, :NW],
                            lhsT=msa[:, m * 128:(m + 1) * 128].bitcast(F32R),
                            rhs=q1[:, j * NW:(j + 1) * NW].bitcast(F32R),
                            start=True, stop=True)
                    u = wkA.tile([128, HW], F32, tag="u")
                    nc.scalar.activation(
                        u.rearrange("p (j n) -> p j n", n=NW), ps[:, :, :NW],
                        ACT.Copy)
                    m8 = wkA.tile([128, 8], F32, tag="m8")
                    i8 = wkA.tile([128, 8], U32, tag="i8")
                    nc.vector.max(m8, u)
                    nc.vector.max_index(i8, m8, u)
                    idxf = wkA.tile([128, 1], F32, tag="idxf")
                    nc.vector.tensor_copy(idxf, i8[:, 0:1])
                    xm = wkA.tile([128, 1], F32, tag="xm")
                    ym = wkA.tile([128, 1], F32, tag="ym")
                    jnk = wkA.tile([128, H - 1], F32, tag="jnk")
                    # y = #{k in 1..31 : k*W <= idx} = idx // W
                    nc.vector.tensor_scalar(jnk, thr56, idxf, None, op0=ALU.is_le,
                                            op1=ALU.add, accum_out=ym)
                    # x = idx - W*y
                    nc.vector.scalar_tensor_tensor(xm, ym, -float(W), idxf,
                                                   op0=ALU.mult, op1=ALU.add)
                    nc.scalar.activation(ycg_t[:, m:m + 1], ym, ACT.Copy,
                                         scale=CG)
                    nc.scalar.activation(xcg_t[:, m:m + 1], xm, ACT.Copy,
                                         scale=CG)
                    # alpha = a8 + (y^2 + x^2)/GD = a8 + ((y*cg)^2+(x*cg)^2)/2
                    ysq = wkA.tile([128, 1], F32, tag="ysq")
                    nc.vector.tensor_mul(ysq, ycg_t[:, m:m + 1], ycg_t[:, m:m + 1])
                    xsq = wkA.tile([128, 1], F32, tag="xsq")
                    nc.vector.tensor_mul(xsq, xcg_t[:, m:m + 1], xcg_t[:, m:m + 1])
                    ssum = wkA.tile([128, 1], F32, tag="ssum")
                    nc.vector.tensor_add(ssum, ysq, xsq)
                    hsum = wkA.tile([128, 1], F32, tag="hsum")
                    nc.scalar.activation(hsum, ssum, ACT.Copy, scale=0.5)
                    nc.vector.tensor_add(alp_t[:, m:m + 1], hsum, a8t[:, m:m + 1])
                    for row, srct in ((64, ycg_t), (65, xcg_t), (66, alp_t)):
                        nc.sync.dma_start(
                            out=msb[row:row + 1, m * 128:(m + 1) * 128],
                            in_=srct[:, m:m + 1])

                def b_pair(psAB, j, m):
                    jb = j * NW
                    ps = psAB.tile([128, 512], F32, tag=f"b{m % 2}")
                    nc.tensor.matmul(
                        ps[:, :NW],
                        lhsT=q2[:, m * 128:(m + 1) * 128].bitcast(F32R),
                        rhs=msb[0:67, jb:jb + NW].bitcast(F32R),
                        start=True, stop=True)
                    nc.scalar.activation(sB[:, m, jb:jb + NW], ps[:, :NW],
                                         ACT.Copy)

                def b_tournament(wkB, j):
                    # segmax for all 14 query chunks of this column block:
                    # 4-round pairwise max tournament on Pool (segmented
                    # tensor_reduce is DVE-only and DVE is saturated by A)
                    jb = j * NW
                    cur = sB[:, :, jb:jb + NW]
                    w = NW
                    for r in range(4):
                        w //= 2
                        halves = cur.rearrange("p m (s k) -> p m s k", k=2)
                        if r < 3:
                            nxt = wkB.tile([128, NCH, w], F32, tag=f"t{r}")
                        else:
                            nxt = seg_all[:, :, j * (NW // SEG):
                                          (j + 1) * (NW // SEG)]
                        nc.gpsimd.scalar_tensor_tensor(
                            nxt, halves[:, :, :, 0], 1.0, halves[:, :, :, 1],
                            op0=ALU.mult, op1=ALU.max)
                        cur = nxt

                with tc.tile_pool(name="psAB", bufs=1, space="PSUM") as psAB, \
                     tc.tile_pool(name="wkA", bufs=3) as wkA, \
                     tc.tile_pool(name="wkB", bufs=1) as wkB:
                    # block j's msb columns are complete after a_chunk(AJ[j]);
                    # spread the 56 (j,m) B matmul+copy pairs a few per A
                    # chunk so they never clog the ACT queue ahead of A's
                    # own psum->sbuf copies
                    AJ = [3, 6, 10, 13]
                    bq = [(j, bm) for j in range(NB) for bm in range(NCH)]
                    ptr = 0
                    for m in range(NCH):
                        a_chunk(psAB, wkA, m)
                        quota = 6
                        while quota > 0 and ptr < len(bq):
                            j, bm = bq[ptr]
                            if AJ[j] > m:
                                break
                            b_pair(psAB, j, bm)
                            ptr += 1
                            quota -= 1
                            if bm == NCH - 1:
                                b_tournament(wkB, j)
                    while ptr < len(bq):
                        j, bm = bq[ptr]
                        b_pair(psAB, j, bm)
                        ptr += 1
                        if bm == NCH - 1:
                            b_tournament(wkB, j)
                for m in range(NCH):
                    nc.sync.dma_start(out=segmax_l[m * 128:(m + 1) * 128, :],
                                      in_=seg_all[:, m, :])

                if PH < 3:
                    raise _Trunc()
                nc.gpsimd.collective_compute(
                    "AllToAll", ALU.bypass, replica_groups=groups,
                    ins=[segmax_l[:]], outs=[segmax_x[:]])

                # rank-51 of global segmaxes, for my 224 queries
                with tc.tile_pool(name="wkT", bufs=2) as wkT:
                    for ci, (p0, pc) in enumerate(((0, 128), (128, 96))):
                        ext = wkT.tile([pc, ND * NSEG], F32, tag=f"ext{ci}")
                        src = segmax_x[:].rearrange("(d p) s -> p d s", d=ND)
                        nc.sync.dma_start(
                            out=ext.rearrange("p (d s) -> p d s", d=ND),
                            in_=src[p0:p0 + pc, :, :])
                        m8 = wkT.tile([pc, 8], F32, tag=f"m8{ci}")
                        for r in range(7):
                            nc.vector.max(m8, ext)
                            if r < 6:
                                nc.vector.match_replace(ext, m8, ext, NEG)
                        nc.sync.dma_start(out=tstats_l[p0:p0 + pc, 0:1],
                                          in_=m8[:, 2:3])

                nc.gpsimd.collective_compute(
                    "AllGather", ALU.bypass, replica_groups=groups,
                    ins=[tstats_l[:]], outs=[tstats_g[:]])

                nc.sync.dma_start(
                    out=t_all[:],
                    in_=tstats_g[:, 0:1].rearrange("(m q) s -> q (m s)", q=128))

                if PH < 4:
                    raise _Trunc()
                # local survivor count + 8 smallest survivors per query
                # (count/maskneg alternate DVE and Pool so the two element
                # scans run on both vector engines concurrently)
                with tc.tile_pool(name="wkC", bufs=3) as wkC:
                    for m in range(NCH):
                        eng = nc.vector if m % 2 == 0 else nc.gpsimd
                        msk = wkC.tile([128, ML], F32, tag="msk")
                        eng.tensor_scalar(
                            msk, sB[:, m, :], t_all[:, m:m + 1], None,
                            op0=ALU.is_lt, op1=ALU.add,
                            accum_out=cnt_t[:, m:m + 1])
                        r = wkC.tile([128, ML], F32, tag="r")
                        eng.scalar_tensor_tensor(
                            r, msk, -BIG, sB[:, m, :], op0=ALU.mult, op1=ALU.subtract)
                        mn = wkC.tile([128, 8], F32, tag="mn")
                        nc.vector.max(mn, r)
                        nc.vector.tensor_scalar(mins_t[:, m, :], mn, -1.0, None,
                                                op0=ALU.mult)
                        nc.sync.dma_start(out=stats_l[m * 128:(m + 1) * 128, 0:1],
                                          in_=cnt_t[:, m:m + 1])
                        nc.sync.dma_start(out=stats_l[m * 128:(m + 1) * 128, 1:9],
                                          in_=mins_t[:, m, :])

            nc.gpsimd.collective_compute(
                "AllGather", ALU.bypass, replica_groups=groups,
                ins=[stats_l[:]], outs=[stats_g[:]])

            if PH < 5:
                raise _Trunc()
            # global boundary: v50/v51 -> tau, redundant on every core
            with tc.tile_pool(name="wkD", bufs=3) as wkD:
                for m in range(NCH):
                    eng = nc.vector if m % 2 == 0 else nc.gpsimd
                    mins64 = wkD.tile([128, ND, 8], F32, tag="m64")
                    nc.sync.dma_start(
                        out=mins64,
                        in_=stats_g[:, m * 128:(m + 1) * 128, 1:9]
                        .rearrange("d p s -> p d s"))
                    cnt8 = wkD.tile([128, ND], F32, tag="c8")
                    nc.sync.dma_start(
                        out=cnt8,
                        in_=stats_g[:, m * 128:(m + 1) * 128, 0:1]
                        .rearrange("d p s -> p (d s)"))
                    cl = wkD.tile([128, 1], F32, tag="cl")
                    nc.vector.tensor_reduce(cl, cnt8, axis=AX.X, op=ALU.add)
                    e = wkD.tile([128, 1], F32, tag="e")
                    # e = (THW - cnt_lt_total) - 50
                    eng.tensor_scalar(e, cl, -1.0, float(THW - 50),
                                      op0=ALU.mult, op1=ALU.add)
                    neg64 = wkD.tile([128, ND * 8], F32, tag="n64")
                    eng.tensor_scalar(
                        neg64, mins64.rearrange("p d s -> p (d s)"), -1.0, None,
                        op0=ALU.mult)
                    asc16 = wkD.tile([128, 16], F32, tag="a16")
                    a8a = wkD.tile([128, 8], F32, tag="a8a")
                    nc.vector.max(a8a, neg64)
                    eng.tensor_scalar(asc16[:, 0:8], a8a, -1.0, None,
                                      op0=ALU.mult)
                    nc.vector.match_replace(neg64, a8a, neg64, NEG)
                    nc.vector.max(a8a, neg64)
                    eng.tensor_scalar(asc16[:, 8:16], a8a, -1.0, None,
                                      op0=ALU.mult)
                    # tau = (asc16[e] + asc16[e-1])/2 selected in one pass:
                    # mask entries j in {e-1, e} via |j - (e-0.5)| <= 0.6
                    em05 = wkD.tile([128, 1], F32, tag="em05")
                    eng.tensor_scalar(em05, e, 0.5, None, op0=ALU.subtract)
                    mk16 = wkD.tile([128, 16], F32, tag="mk16")
                    eng.tensor_scalar(mk16, iota16, em05, 0.0,
                                      op0=ALU.subtract, op1=ALU.abs_max)
                    mk2 = wkD.tile([128, 16], F32, tag="mk2")
                    eng.tensor_scalar(mk2, mk16, 0.6, None, op0=ALU.is_le)
                    junk = wkD.tile([128, 16], F32, tag="junk")
                    vsum = wkD.tile([128, 1], F32, tag="vsum")
                    eng.scalar_tensor_tensor(
                        junk, asc16, 1.0, mk2, op0=ALU.mult, op1=ALU.mult,
                        accum_out=vsum)
                    eng.tensor_scalar(tauc_t[:, m:m + 1], vsum, 0.5, None,
                                      op0=ALU.mult)

            # tau (absolute) becomes q2c channel 67: psC = s_hat - tau, so the
            # top-50 mask is just sign(psC) and exp(psC) is range-safe.
            part_to_row(scr[3], q2c[67:68, :], tauc_t[:])

            if PH < 6:
                raise _Trunc()
            # ---------------- phase C: weights + readout ----------------
            with tc.tile_pool(name="Wpool", bufs=1) as Wpool:
                Wt = Wpool.tile([128, NCH, ML], BF16)
                with tc.tile_pool(name="psC", bufs=2, space="PSUM") as psC, \
                     tc.tile_pool(name="wkF", bufs=3) as wkF:
                    for k in range(NCH):
                        ps = psC.tile([128, NB, 512], F32)
                        for j in range(NB):
                            nc.tensor.matmul(
                                ps[:, j, :NW],
                                lhsT=msb[:, k * 128:(k + 1) * 128].bitcast(F32R),
                                rhs=q2c[:, j * NW:(j + 1) * NW].bitcast(F32R),
                                start=True, stop=True)
                        # psC = s - tau: z = min(ps*BIG, ps) maps rejected
                        # entries (ps<0) to -huge so exp(z) = masked weight;
                        # DVE and Pool each premask half the chunk in parallel
                        z = wkF.tile([128, NB, NW], F32, tag="z")
                        nc.vector.scalar_tensor_tensor(
                            z[:, 0:2], ps[:, 0:2, :NW], BIG, ps[:, 0:2, :NW],
                            op0=ALU.mult, op1=ALU.min)
                        nc.gpsimd.scalar_tensor_tensor(
                            z[:, 2:4], ps[:, 2:4, :NW], BIG, ps[:, 2:4, :NW],
                            op0=ALU.mult, op1=ALU.min)
                        nc.scalar.activation(
                            Wt[:, k, :].rearrange("p (j n) -> p j n", n=NW),
                            z, ACT.Exp)

                if PH < 7:
                    raise _Trunc()
                # Readout in two k-halves with SBUF partial accumulation so
                # the first half's matmuls overlap phase C's tail. The norm
                # row accumulates across both halves in a persistent PSUM
                # tile (4 banks) while value chunks cycle the other 4 banks.
                KH = NCH // 2
                with tc.tile_pool(name="wkO", bufs=2) as wkO, \
                     tc.tile_pool(name="accp", bufs=1) as accp:
                    pacc = accp.tile([128, MCV, NB, NW], F32)
                    with tc.tile_pool(name="psO", bufs=2, space="PSUM") as psO:
                        for half in range(2):
                            k0 = half * KH
                            for mc in range(MCV):
                                po = psO.tile([128, NB, 512], F32, tag="po")
                                for k in range(k0, k0 + KH):
                                    for j in range(NB):
                                        nc.tensor.matmul(
                                            po[:, j, :NW],
                                            lhsT=mvt3[:, k,
                                                      mc * 128:(mc + 1) * 128],
                                            rhs=Wt[:, k, j * NW:(j + 1) * NW],
                                            start=(k == k0),
                                            stop=(k == k0 + KH - 1))
                                if half == 0:
                                    nc.scalar.activation(
                                        pacc[:, mc], po[:, :, :NW], ACT.Copy)
                                else:
                                    ob = wkO.tile([128, NB, NW], F32, tag="ob")
                                    nc.vector.tensor_add(
                                        ob, pacc[:, mc], po[:, :, :NW])
                                    # value rows v=64d+i land at ro row 65d+i
                                    for h in range(2):
                                        d = 2 * mc + h
                                        nc.sync.dma_start(
                                            out=ro_l[d * 65:d * 65 + 64, :]
                                            .rearrange("r (j n) -> r j n", n=NW),
                                            in_=ob[h * 64:(h + 1) * 64])
                    with tc.tile_pool(name="psN", bufs=1, space="PSUM") as psN:
                        pn = psN.tile([1, NB, 512], F32)
                        for k in range(NCH):
                            for j in range(NB):
                                nc.tensor.matmul(
                                    pn[:, j, :NW], lhsT=ones_cb,
                                    rhs=Wt[:, k, j * NW:(j + 1) * NW],
                                    start=(k == 0), stop=(k == NCH - 1))
                        nb_ = wkO.tile([1, NB, NW], F32, tag="nb")
                        nc.scalar.activation(nb_, pn[:, :, :NW], ACT.Copy)
                        for d in range(ND):
                            nc.sync.dma_start(
                                out=ro_l[d * 65 + 64:d * 65 + 65, :]
                                .rearrange("r (j n) -> r j n", n=NW),
                                in_=nb_)

            if PH < 8:
                raise _Trunc()
            # one ReduceScatter delivers 64 summed value rows + the summed
            # norm row to each core; the host does the division.
            nc.gpsimd.collective_compute(
                "ReduceScatter", ALU.add, replica_groups=groups,
                ins=[ro_l[:]], outs=[out_d[:]])

        except _Trunc:
            pass
    if not nc.is_finalized():
        nc.finalize()
    return nc


def _host_inputs(mk, qk, mv):
    mkf = np.asarray(mk, np.float32).reshape(CK, THW)
    qkf = np.asarray(qk, np.float32).reshape(CK, HW)
    mvf = np.asarray(mv, np.float32).reshape(CV, THW)
    c = (qkf * qkf).sum(0)
    a = (mkf * mkf).sum(0)
    yv = (np.arange(HW, dtype=np.float32) // W)
    xv = (np.arange(HW, dtype=np.float32) % W)

    q1 = np.empty((65, HW), np.float32)
    q1[0] = c / 8.0
    q1[1:65] = qkf
    q2 = np.empty((67, HW), np.float32)
    q2[0:64] = qkf
    q2[64] = yv * CG
    q2[65] = xv * CG
    q2[66] = -1.0
    q2c = np.empty((68, HW), np.float32)
    q2c[0:64] = qkf
    q2c[64] = yv * CG
    q2c[65] = xv * CG
    q2c[66] = -1.0
    q2c[67] = 0.0

    in_maps = []
    for d in range(ND):
        sl = slice(d * ML, (d + 1) * ML)
        msa = np.zeros((65, ML), np.float32)
        msa[0] = -1.0
        msa[1:65] = mkf[:, sl] / 4.0
        msb = np.zeros((68, ML), np.float32)
        msb[0:64] = mkf[:, sl] / 4.0
        msb[67] = -1.0
        a8t = np.ascontiguousarray(
            (a[sl] / 8.0).reshape(NCH, 128).T.astype(np.float32))
        mvt = np.ascontiguousarray(
            mvf[:, sl].T.reshape(NCH, 128, CV).transpose(1, 0, 2)
            .reshape(128, NCH * CV)).astype(ml_dtypes.bfloat16)
        in_maps.append({
            "msa": msa, "msb": msb, "q1": q1, "q2": q2, "q2c": q2c,
            "a8t": a8t, "mvt": mvt,
        })
    return in_maps


_NC_CACHE = {}


def _get_nc():
    if "nc" not in _NC_CACHE:
        _NC_CACHE["nc"] = _build()
    return _NC_CACHE["nc"]


def assemble(per_core_outs):
    """Each core returns [65, HW]: 64 summed value rows + the summed norm
    row. Normalize host-side and concatenate the 8 slices."""
    parts = []
    for o in per_core_outs:
        o = np.asarray(o, np.float32)
        parts.append(o[0:CV // ND] / o[CV // ND:CV // ND + 1])
    return np.concatenate(parts, axis=0).reshape(1, CV, H, W)


def kernel(mk, qk, mv):
    from concourse.bass_utils import run_bass_kernel_spmd
    in_maps = _host_inputs(mk, qk, mv)
    nc = _get_nc()
    res = run_bass_kernel_spmd(nc, in_maps, core_ids=list(range(ND)))
    return assemble([res.results[d]["out"] for d in range(ND)])



# revision 34
# speedup vs baseline: 1.0905x; 1.0905x over previous
"""Trainium2 Bass kernel for EvalMemoryReader (retrieval_knn).

Distributed plan (8 NeuronCores):
  A. memory-sharded argmax: fp32r matmul (own 1792 memory rows x all 1792
     queries) -> per-row argmax via DVE max8+find-index -> gaussian center
     (ym, xm) and alpha per memory row.  One AllGather ships the three
     gaussian rows for all 14336 memory rows to every core.
  B. query-sharded selection: each core computes scores s(m, q) for its own
     224 queries over ALL 14336 memory rows (fp32r matmul, 67 channels
     folding the gaussian), takes segment-16 maxima via a Pool max
     tournament, rank-51 of the 896 segmaxes via 7x(max8+match_replace)
     -> threshold t, then an in-place 4-pass sweep over the score row
     (v = t-s, z = min(v, -BIG*v), max8 -> 8 smallest survivors, count)
     -> exact v50/v51 midpoint tau per query.  AllGather tau (tiny).
  C. memory-sharded weights: fp32r matmul with tau folded in as a channel
     (psum = s - tau), premask z = min(ps*BIG, ps), exp -> bf16 weights;
     bf16 readout matmul in two k-halves + a norm row; one ReduceScatter
     with 65-row interleaving delivers summed values + norm; host divides.

kernel() takes FULL inputs, shards host-side, runs SPMD on cores 0-7.
"""

import math
import os

import ml_dtypes
import numpy as np

import concourse.bass as bass
import concourse.bacc as bacc
import concourse.mybir as mybir
from concourse.tile import TileContext

ND = 8
CK, CV, T, H, W = 64, 512, 8, 32, 56
HW = H * W              # 1792 queries
THW = T * HW            # 14336 memory locations
ML = THW // ND          # 1792 memory rows per core
NCH = HW // 128         # 14 chunks of 128
NB = 4                  # 448-wide free-dim chunks per 1792
NW = HW // NB           # 448
SEG = 16
NSEG = THW // SEG       # 896 segments per query (global)
NQ = HW // ND           # 224 queries per core
NJ = THW // NW          # 32 column blocks of 448 in the selection matmul
GD = 2.0 * 5.6 * 5.6    # 62.72
CG = math.sqrt(2.0 / GD)
BIG = 1.0e30
NEG = -1.0e30
MCV = CV // 128         # 4 output chunks

F32 = mybir.dt.float32
F32R = mybir.dt.float32r
BF16 = mybir.dt.bfloat16
U32 = mybir.dt.uint32
ALU = mybir.AluOpType
ACT = mybir.ActivationFunctionType
AX = mybir.AxisListType


class _Trunc(Exception):
    pass


def _build():
    nc = bacc.Bacc(num_devices=ND)

    # msb rows: 0-63 own mk/4, 64-66 gaussian rows (runtime), 67 = -1 (tau
    # channel for phase C), 68 = -1 (|q|^2 channel for phase A)
    msb_d = nc.dram_tensor("msb", [69, ML], F32, kind="ExternalInput")
    # q2c rows: 0-63 qk, 64 yv*cg, 65 xv*cg, 66 = -1, 67 = tau (runtime),
    # 68 = |q|^2/8
    q2c_d = nc.dram_tensor("q2c", [69, HW], F32, kind="ExternalInput")
    # selection operands: full-memory channels + own-query columns
    mba_d = nc.dram_tensor("mba", [67, THW], F32, kind="ExternalInput")
    q2o_d = nc.dram_tensor("q2o", [67, NQ], F32, kind="ExternalInput")
    a8t_d = nc.dram_tensor("a8t", [128, NCH], F32, kind="ExternalInput")
    mvt_d = nc.dram_tensor("mvt", [128, NCH * CV], BF16, kind="ExternalInput")
    out_d = nc.dram_tensor("out", [CV // ND + 1, HW], F32, kind="ExternalOutput")

    iota16_c = nc.inline_tensor(
        np.broadcast_to(np.arange(16, dtype=np.float32), (128, 16)).copy(),
        name="iota16")
    ones_128x1_c = nc.inline_tensor(
        np.ones((128, 1), np.float32).astype(ml_dtypes.bfloat16), name="o128x1")
    thr56_c = nc.inline_tensor(
        np.broadcast_to(np.arange(1, H, dtype=np.float32) * W, (128, H - 1))
        .copy(), name="thr56")

    # collective bounce buffers
    gau_l = nc.dram_tensor("gau_l", [3, ML], F32)
    gau_g = nc.dram_tensor("gau_g", [ND, 3, ML], F32, addr_space="Shared")
    tau_l = nc.dram_tensor("tau_l", [NQ, 1], F32)
    tau_g = nc.dram_tensor("tau_g", [HW, 1], F32, addr_space="Shared")
    scr = [nc.dram_tensor(f"scr{i}", [HW], F32) for i in range(3)]
    # readout rows interleaved in groups of 65 per core: rows 65d..65d+63 are
    # value rows 64d..64d+63, row 65d+64 is a copy of the local norm row, so a
    # single ReduceScatter delivers each core its value slice + global norm.
    ro_l = nc.dram_tensor("ro_l", [(CV // ND + 1) * ND, HW], F32)

    groups = [list(range(ND))]

    from contextlib import ExitStack
    with TileContext(nc) as tc, ExitStack() as es:
        try:
            POOL_E = mybir.EngineType.Pool
            cpool = es.enter_context(tc.tile_pool(name="consts", bufs=1))
            def cload(ap, name):
                return cpool.tile_from(ap, force_copy=True, name=name,
                                       forced_dma_engine=POOL_E)
            msb = cload(msb_d[:], "msb_t")
            q2c = cload(q2c_d[:], "q2c_t")
            mba = cload(mba_d[:], "mba_t")
            q2o = cload(q2o_d[:], "q2o_t")
            a8t = cload(a8t_d[:], "a8t_t")
            iota16 = cload(iota16_c[:], "iota16_t")
            ones_cb = cload(ones_128x1_c[:], "ones_cb_t")
            thr56 = cload(thr56_c[:], "thr56_t")

            spool = es.enter_context(tc.tile_pool(name="smalls", bufs=1))
            ycg_t = spool.tile([128, NCH], F32)
            xcg_t = spool.tile([128, NCH], F32)
            alp_t = spool.tile([128, NCH], F32)

            def part_to_row(scratch, row_ap, tile_ap):
                nc.sync.dma_start(
                    out=scratch[:].rearrange("(m q) -> q m", q=128), in_=tile_ap)
                nc.sync.dma_start(out=row_ap, in_=scratch[:])

            PH = int(os.environ.get("KPHASE", "99"))

            # ---------------- phase A: argmax per memory row ----------------
            with tc.tile_pool(name="psA", bufs=2, space="PSUM") as psA, \
                 tc.tile_pool(name="wkA", bufs=3) as wkA:
                for m in range(NCH):
                    ps = psA.tile([128, NB, 512], F32)
                    for j in range(NB):
                        nc.tensor.matmul(
                            ps[:, j, :NW],
                            lhsT=msb[0:69, m * 128:(m + 1) * 128].bitcast(F32R),
                            rhs=q2c[0:69, j * NW:(j + 1) * NW].bitcast(F32R),
                            start=True, stop=True)
                    u = wkA.tile([128, HW], F32, tag="u")
                    nc.scalar.activation(
                        u.rearrange("p (j n) -> p j n", n=NW), ps[:, :, :NW],
                        ACT.Copy)
                    m8 = wkA.tile([128, 8], F32, tag="m8")
                    i8 = wkA.tile([128, 8], U32, tag="i8")
                    nc.vector.max(m8, u)
                    nc.vector.max_index(i8, m8, u)
                    idxf = wkA.tile([128, 1], F32, tag="idxf")
                    nc.vector.tensor_copy(idxf, i8[:, 0:1])
                    xm = wkA.tile([128, 1], F32, tag="xm")
                    ym = wkA.tile([128, 1], F32, tag="ym")
                    jnk = wkA.tile([128, H - 1], F32, tag="jnk")
                    # y = #{k in 1..31 : k*W <= idx} = idx // W
                    nc.vector.tensor_scalar(jnk, thr56, idxf, None, op0=ALU.is_le,
                                            op1=ALU.add, accum_out=ym)
                    # x = idx - W*y
                    nc.vector.scalar_tensor_tensor(xm, ym, -float(W), idxf,
                                                   op0=ALU.mult, op1=ALU.add)
                    nc.scalar.activation(ycg_t[:, m:m + 1], ym, ACT.Copy,
                                         scale=CG)
                    nc.scalar.activation(xcg_t[:, m:m + 1], xm, ACT.Copy,
                                         scale=CG)
                    # alpha = a8 + (y^2 + x^2)/GD = a8 + ((y*cg)^2+(x*cg)^2)/2
                    ysq = wkA.tile([128, 1], F32, tag="ysq")
                    nc.vector.tensor_mul(ysq, ycg_t[:, m:m + 1], ycg_t[:, m:m + 1])
                    xsq = wkA.tile([128, 1], F32, tag="xsq")
                    nc.vector.tensor_mul(xsq, xcg_t[:, m:m + 1], xcg_t[:, m:m + 1])
                    ssum = wkA.tile([128, 1], F32, tag="ssum")
                    nc.vector.tensor_add(ssum, ysq, xsq)
                    hsum = wkA.tile([128, 1], F32, tag="hsum")
                    nc.scalar.activation(hsum, ssum, ACT.Copy, scale=0.5)
                    nc.vector.tensor_add(alp_t[:, m:m + 1], hsum, a8t[:, m:m + 1])
                    # stream this chunk's gaussian channels into msb rows
                    # 64..66 for phase C
                    for row, srct in ((64, ycg_t), (65, xcg_t), (66, alp_t)):
                        nc.sync.dma_start(
                            out=msb[row:row + 1, m * 128:(m + 1) * 128],
                            in_=srct[:, m:m + 1])

            # ship the gaussian rows for all memory rows to every core
            part_to_row(scr[0], gau_l[0:1, :], ycg_t[:])
            part_to_row(scr[1], gau_l[1:2, :], xcg_t[:])
            part_to_row(scr[2], gau_l[2:3, :], alp_t[:])
            if PH < 2:
                raise _Trunc()
            nc.gpsimd.collective_compute(
                "AllGather", ALU.bypass, replica_groups=groups,
                ins=[gau_l[:]], outs=[gau_g[:]])
            for r in range(3):
                nc.sync.dma_start(
                    out=mba[64 + r:65 + r, :].rearrange("a (d m) -> a d m",
                                                        d=ND),
                    in_=gau_g[:, r:r + 1, :].rearrange("d a m -> a d m"))

            if PH < 3:
                raise _Trunc()
            # -------- phase B: query-sharded selection over all memory ------
            with tc.tile_pool(name="sSpool", bufs=1) as sSpool, \
                 tc.tile_pool(name="wkB", bufs=1) as wkB, \
                 tc.tile_pool(name="wkT", bufs=2) as wkT, \
                 tc.tile_pool(name="psB", bufs=1, space="PSUM") as psB:
                sS = sSpool.tile([128, 2, THW], F32)
                seg = sSpool.tile([128, 2, NSEG], F32)
                for ci, (p0, pc) in enumerate(((0, 128), (128, 96))):
                    for j in range(NJ):
                        ps = psB.tile([128, 512], F32, tag=f"b{j % 4}")
                        nc.tensor.matmul(
                            ps[0:pc, :NW],
                            lhsT=q2o[:, p0:p0 + pc].bitcast(F32R),
                            rhs=mba[:, j * NW:(j + 1) * NW].bitcast(F32R),
                            start=True, stop=True)
                        nc.scalar.activation(
                            sS[0:pc, ci, j * NW:(j + 1) * NW], ps[0:pc, :NW],
                            ACT.Copy)
                    # segment-16 maxima via 4-round pairwise max tournament on
                    # Pool, batched over groups of 4 column blocks
                    for g in range(NJ // 4):
                        cur = sS[0:pc, ci, g * 4 * NW:(g + 1) * 4 * NW] \
                            .rearrange("p (b n) -> p b n", b=4)
                        w = NW
                        for r in range(4):
                            w //= 2
                            halves = cur.rearrange("p b (s k) -> p b s k", k=2)
                            if r < 3:
                                nxtf = wkB.tile([128, 4, w], F32, tag=f"t{r}", name=f"tt{r}")
                                nxt = nxtf[0:pc]
                            else:
                                nxt = seg[0:pc, ci,
                                          g * 112:(g + 1) * 112] \
                                    .rearrange("p (b s) -> p b s", b=4)
                            nc.gpsimd.scalar_tensor_tensor(
                                nxt, halves[:, :, :, 0], 1.0,
                                halves[:, :, :, 1], op0=ALU.mult, op1=ALU.max)
                            cur = nxt

                    if PH < 4:
                        raise _Trunc()
                    # rank-51 of the 896 segmaxes -> provisional threshold t
                    ext = seg[0:pc, ci, :]
                    m8f = wkT.tile([128, 8], F32, tag=f"m8{ci}", name="n"+f"m8{ci}")
                    m8 = m8f[0:pc]
                    for r in range(7):
                        nc.vector.max(m8, ext)
                        if r < 6:
                            nc.vector.match_replace(ext, m8, ext, NEG)
                    t_cf = wkT.tile([128, 1], F32, tag=f"t{ci}", name="n"+f"t{ci}")
                    t_c = t_cf[0:pc]
                    nc.vector.tensor_copy(t_c, m8[:, 2:3])

                    # in-place sweep over the 14336-wide score row, split
                    # half/half across DVE and Pool:
                    #   v = t - s ; z = min(v, -BIG*v) ; max8 -> 8 smallest
                    #   survivors (as t - s, descending) ; count survivors
                    S = sS[0:pc, ci, :]
                    HWF = THW // 2
                    for eng, sl in ((nc.vector, slice(0, HWF)),
                                    (nc.gpsimd, slice(HWF, THW))):
                        eng.tensor_scalar(S[:, sl], S[:, sl], t_c, -1.0,
                                          op0=ALU.subtract, op1=ALU.mult)
                    for eng, sl in ((nc.vector, slice(0, HWF)),
                                    (nc.gpsimd, slice(HWF, THW))):
                        eng.scalar_tensor_tensor(
                            S[:, sl], S[:, sl], -BIG, S[:, sl],
                            op0=ALU.mult, op1=ALU.min)
                    mn8f = wkT.tile([128, 8], F32, tag=f"mn{ci}", name="n"+f"mn{ci}")
                    mn8 = mn8f[0:pc]
                    nc.vector.max(mn8, S)
                    cAf = wkT.tile([128, 1], F32, tag=f"cA{ci}", name="n"+f"cA{ci}")
                    cA = cAf[0:pc]
                    cBf = wkT.tile([128, 1], F32, tag=f"cB{ci}", name="n"+f"cB{ci}")
                    cB = cBf[0:pc]
                    for eng, sl, cacc in ((nc.vector, slice(0, HWF), cA),
                                          (nc.gpsimd, slice(HWF, THW), cB)):
                        eng.tensor_scalar(S[:, sl], S[:, sl], -1.0e20, None,
                                          op0=ALU.is_ge, op1=ALU.add,
                                          accum_out=cacc)
                    # tau = t - (mn8[e] + mn8[e-1])/2 with e = count - 50
                    em05f = wkT.tile([128, 1], F32, tag=f"e{ci}", name="n"+f"e{ci}")
                    em05 = em05f[0:pc]
                    nc.vector.tensor_add(em05, cA, cB)
                    nc.vector.tensor_scalar(em05, em05, 50.5, None,
                                            op0=ALU.subtract)
                    d8f = wkT.tile([128, 8], F32, tag=f"d8{ci}", name="n"+f"d8{ci}")
                    d8 = d8f[0:pc]
                    nc.vector.tensor_scalar(d8, iota16[0:pc, 0:8], em05, 0.0,
                                            op0=ALU.subtract, op1=ALU.abs_max)
                    mk2f = wkT.tile([128, 8], F32, tag=f"mk{ci}", name="n"+f"mk{ci}")
                    mk2 = mk2f[0:pc]
                    nc.vector.tensor_scalar(mk2, d8, 0.6, None, op0=ALU.is_le)
                    junk8f = wkT.tile([128, 8], F32, tag=f"j8{ci}", name="n"+f"j8{ci}")
                    junk8 = junk8f[0:pc]
                    msumf = wkT.tile([128, 1], F32, tag=f"ms{ci}", name="n"+f"ms{ci}")
                    msum = msumf[0:pc]
                    nc.vector.scalar_tensor_tensor(
                        junk8, mn8, 1.0, mk2, op0=ALU.mult, op1=ALU.mult,
                        accum_out=msum)
                    tau_cf = wkT.tile([128, 1], F32, tag=f"tv{ci}", name="n"+f"tv{ci}")
                    tau_c = tau_cf[0:pc]
                    nc.vector.scalar_tensor_tensor(
                        tau_c, msum, -0.5, t_c, op0=ALU.mult, op1=ALU.add)
                    nc.sync.dma_start(out=tau_l[p0:p0 + pc, :], in_=tau_c)

            if PH < 5:
                raise _Trunc()
            nc.gpsimd.collective_compute(
                "AllGather", ALU.bypass, replica_groups=groups,
                ins=[tau_l[:]], outs=[tau_g[:]])
            # tau (absolute) becomes q2c channel 67: psC = s - tau
            nc.sync.dma_start(out=q2c[67:68, :],
                              in_=tau_g[:].rearrange("q s -> s q"))

            if PH < 6:
                raise _Trunc()
            # ---------------- phase C: weights + readout ----------------
            with tc.tile_pool(name="Wpool", bufs=1) as Wpool, \
                 tc.tile_pool(name="mvp", bufs=1) as mvpool:
                Wt = Wpool.tile([128, NCH, ML], BF16)
                mvt = mvpool.tile_from(mvt_d[:], force_copy=True,
                                       forced_dma_engine=POOL_E)
                mvt3 = mvt.rearrange("p (k c) -> p k c", c=CV)
                with tc.tile_pool(name="psC", bufs=2, space="PSUM") as psC, \
                     tc.tile_pool(name="wkF", bufs=3) as wkF:
                    for k in range(NCH):
                        ps = psC.tile([128, NB, 512], F32)
                        for j in range(NB):
                            nc.tensor.matmul(
                                ps[:, j, :NW],
                                lhsT=msb[0:68, k * 128:(k + 1) * 128]
                                .bitcast(F32R),
                                rhs=q2c[0:68, j * NW:(j + 1) * NW]
                                .bitcast(F32R),
                                start=True, stop=True)
                        # psC = s - tau: z = min(ps*BIG, ps) maps rejected
                        # entries (ps<0) to -huge so exp(z) = masked weight;
                        # DVE and Pool each premask half the chunk
                        z = wkF.tile([128, NB, NW], F32, tag="z")
                        nc.vector.scalar_tensor_tensor(
                            z[:, 0:2], ps[:, 0:2, :NW], BIG, ps[:, 0:2, :NW],
                            op0=ALU.mult, op1=ALU.min)
                        nc.gpsimd.scalar_tensor_tensor(
                            z[:, 2:4], ps[:, 2:4, :NW], BIG, ps[:, 2:4, :NW],
                            op0=ALU.mult, op1=ALU.min)
                        nc.scalar.activation(
                            Wt[:, k, :].rearrange("p (j n) -> p j n", n=NW),
                            z, ACT.Exp)

                if PH < 7:
                    raise _Trunc()
                # Readout in two k-halves with SBUF partial accumulation so
                # the first half's matmuls overlap phase C's tail.
                KH = NCH // 2
                with tc.tile_pool(name="wkO", bufs=2) as wkO, \
                     tc.tile_pool(name="accp", bufs=1) as accp:
                    pacc = accp.tile([128, MCV, NB, NW], F32)
                    with tc.tile_pool(name="psO", bufs=2, space="PSUM") as psO:
                        for half in range(2):
                            k0 = half * KH
                            for mc in range(MCV):
                                po = psO.tile([128, NB, 512], F32, tag="po")
                                for k in range(k0, k0 + KH):
                                    for j in range(NB):
                                        nc.tensor.matmul(
                                            po[:, j, :NW],
                                            lhsT=mvt3[:, k,
                                                      mc * 128:(mc + 1) * 128],
                                            rhs=Wt[:, k, j * NW:(j + 1) * NW],
                                            start=(k == k0),
                                            stop=(k == k0 + KH - 1))
                                if half == 0:
                                    nc.scalar.activation(
                                        pacc[:, mc], po[:, :, :NW], ACT.Copy)
                                else:
                                    ob = wkO.tile([128, NB, NW], F32, tag="ob")
                                    nc.vector.tensor_add(
                                        ob, pacc[:, mc], po[:, :, :NW])
                                    # value rows v=64d+i land at ro row 65d+i
                                    for h in range(2):
                                        d = 2 * mc + h
                                        nc.sync.dma_start(
                                            out=ro_l[d * 65:d * 65 + 64, :]
                                            .rearrange("r (j n) -> r j n",
                                                       n=NW),
                                            in_=ob[h * 64:(h + 1) * 64])
                    with tc.tile_pool(name="psN", bufs=1, space="PSUM") as psN:
                        pn = psN.tile([1, NB, 512], F32)
                        for k in range(NCH):
                            for j in range(NB):
                                nc.tensor.matmul(
                                    pn[:, j, :NW], lhsT=ones_cb,
                                    rhs=Wt[:, k, j * NW:(j + 1) * NW],
                                    start=(k == 0), stop=(k == NCH - 1))
                        nb_ = wkO.tile([1, NB, NW], F32, tag="nb")
                        nc.scalar.activation(nb_, pn[:, :, :NW], ACT.Copy)
                        for d in range(ND):
                            nc.sync.dma_start(
                                out=ro_l[d * 65 + 64:d * 65 + 65, :]
                                .rearrange("r (j n) -> r j n", n=NW),
                                in_=nb_)

            if PH < 8:
                raise _Trunc()
            # one ReduceScatter delivers 64 summed value rows + the summed
            # norm row to each core; the host does the division.
            nc.gpsimd.collective_compute(
                "ReduceScatter", ALU.add, replica_groups=groups,
                ins=[ro_l[:]], outs=[out_d[:]])

        except _Trunc:
            pass
    if not nc.is_finalized():
        nc.finalize()
    return nc


def _host_inputs(mk, qk, mv):
    mkf = np.asarray(mk, np.float32).reshape(CK, THW)
    qkf = np.asarray(qk, np.float32).reshape(CK, HW)
    mvf = np.asarray(mv, np.float32).reshape(CV, THW)
    c = (qkf * qkf).sum(0)
    a = (mkf * mkf).sum(0)
    yv = (np.arange(HW, dtype=np.float32) // W)
    xv = (np.arange(HW, dtype=np.float32) % W)

    q2c = np.zeros((69, HW), np.float32)
    q2c[0:64] = qkf
    q2c[64] = yv * CG
    q2c[65] = xv * CG
    q2c[66] = -1.0
    q2c[67] = 0.0
    q2c[68] = c / 8.0

    mba = np.zeros((67, THW), np.float32)
    mba[0:64] = mkf / 4.0

    in_maps = []
    for d in range(ND):
        sl = slice(d * ML, (d + 1) * ML)
        qsl = slice(d * NQ, (d + 1) * NQ)
        msb = np.zeros((69, ML), np.float32)
        msb[0:64] = mkf[:, sl] / 4.0
        msb[67] = -1.0
        msb[68] = -1.0
        q2o = np.zeros((67, NQ), np.float32)
        q2o[0:64] = qkf[:, qsl]
        q2o[64] = (yv * CG)[qsl]
        q2o[65] = (xv * CG)[qsl]
        q2o[66] = -1.0
        a8t = np.ascontiguousarray(
            (a[sl] / 8.0).reshape(NCH, 128).T.astype(np.float32))
        mvt = np.ascontiguousarray(
            mvf[:, sl].T.reshape(NCH, 128, CV).transpose(1, 0, 2)
            .reshape(128, NCH * CV)).astype(ml_dtypes.bfloat16)
        in_maps.append({
            "msb": msb, "q2c": q2c, "mba": mba, "q2o": q2o,
            "a8t": a8t, "mvt": mvt,
        })
    return in_maps


_NC_CACHE = {}


def _get_nc():
    if "nc" not in _NC_CACHE:
        _NC_CACHE["nc"] = _build()
    return _NC_CACHE["nc"]


def assemble(per_core_outs):
    """Each core returns [65, HW]: 64 summed value rows + the summed norm
    row. Normalize host-side and concatenate the 8 slices."""
    parts = []
    for o in per_core_outs:
        o = np.asarray(o, np.float32)
        parts.append(o[0:CV // ND] / o[CV // ND:CV // ND + 1])
    return np.concatenate(parts, axis=0).reshape(1, CV, H, W)


def kernel(mk, qk, mv):
    from concourse.bass_utils import run_bass_kernel_spmd
    in_maps = _host_inputs(mk, qk, mv)
    nc = _get_nc()
    res = run_bass_kernel_spmd(nc, in_maps, core_ids=list(range(ND)))
    return assemble([res.results[d]["out"] for d in range(ND)])


# revision 39
# speedup vs baseline: 1.1549x; 1.0591x over previous
"""Trainium2 Bass kernel for EvalMemoryReader (retrieval_knn).

Distributed plan (8 NeuronCores):
  A. memory-sharded argmax: fp32r matmul (own 1792 memory rows x all 1792
     queries) -> per-row argmax via DVE max8+find-index -> gaussian center
     (ym, xm) and alpha per memory row.  One AllGather ships the three
     gaussian rows for all 14336 memory rows to every core.
  B. query-sharded selection: each core computes scores s(m, q) for its own
     224 queries over ALL 14336 memory rows (fp32r matmul, 67 channels
     folding the gaussian), takes segment-16 maxima via a Pool max
     tournament, rank-51 of the 896 segmaxes via 7x(max8+match_replace)
     -> threshold t, then an in-place 4-pass sweep over the score row
     (v = t-s, z = min(v, -BIG*v), max8 -> 8 smallest survivors, count)
     -> exact v50/v51 midpoint tau per query.  AllGather tau (tiny).
  C. memory-sharded weights: fp32r matmul with tau folded in as a channel
     (psum = s - tau), premask z = min(ps*BIG, ps), exp -> bf16 weights;
     bf16 readout matmul in two k-halves + a norm row; one ReduceScatter
     with 65-row interleaving delivers summed values + norm; host divides.

kernel() takes FULL inputs, shards host-side, runs SPMD on cores 0-7.
"""

import math
import os

import ml_dtypes
import numpy as np

import concourse.bass as bass
import concourse.bacc as bacc
import concourse.mybir as mybir
from concourse.tile import TileContext

ND = 8
CK, CV, T, H, W = 64, 512, 8, 32, 56
HW = H * W              # 1792 queries
THW = T * HW            # 14336 memory locations
ML = THW // ND          # 1792 memory rows per core
NCH = HW // 128         # 14 chunks of 128
NB = 4                  # 448-wide free-dim chunks per 1792
NW = HW // NB           # 448
SEG = 16
NSEG = THW // SEG       # 896 segments per query (global)
NQ = HW // ND           # 224 queries per core
NJ = THW // NW          # 32 column blocks of 448 in the selection matmul
GD = 2.0 * 5.6 * 5.6    # 62.72
CG = math.sqrt(2.0 / GD)
BIG = 1.0e30
NEG = -1.0e30
MCV = CV // 128         # 4 output chunks

F32 = mybir.dt.float32
F32R = mybir.dt.float32r
BF16 = mybir.dt.bfloat16
U32 = mybir.dt.uint32
ALU = mybir.AluOpType
ACT = mybir.ActivationFunctionType
AX = mybir.AxisListType


class _Trunc(Exception):
    pass


def _build():
    nc = bacc.Bacc(num_devices=ND)

    # msb rows: 0-63 own mk/4, 64-66 gaussian rows (runtime), 67 = -1 (tau
    # channel for phase C), 68 = -1 (|q|^2 channel for phase A)
    msb_d = nc.dram_tensor("msb", [69, ML], F32, kind="ExternalInput")
    # q2c rows: 0-63 qk, 64 yv*cg, 65 xv*cg, 66 = -1, 67 = tau (runtime),
    # 68 = |q|^2/8
    q2c_d = nc.dram_tensor("q2c", [69, HW], F32, kind="ExternalInput")
    # selection operands: full-memory channels + own-query columns
    mba_d = nc.dram_tensor("mba", [67, THW], F32, kind="ExternalInput")
    q2o_d = nc.dram_tensor("q2o", [67, NQ], F32, kind="ExternalInput")
    a8t_d = nc.dram_tensor("a8t", [128, NCH], F32, kind="ExternalInput")
    mvt_d = nc.dram_tensor("mvt", [128, NCH * CV], BF16, kind="ExternalInput")
    out_d = nc.dram_tensor("out", [CV // ND + 1, HW], F32, kind="ExternalOutput")

    iota16_c = nc.inline_tensor(
        np.broadcast_to(np.arange(16, dtype=np.float32), (128, 16)).copy(),
        name="iota16")
    ones_128x1_c = nc.inline_tensor(
        np.ones((128, 1), np.float32).astype(ml_dtypes.bfloat16), name="o128x1")
    thr56_c = nc.inline_tensor(
        np.broadcast_to(np.arange(1, H, dtype=np.float32) * W, (128, H - 1))
        .copy(), name="thr56")
    b20_c = nc.inline_tensor(
        np.full((128, 1), 1.0e20, np.float32), name="b20")

    # collective bounce buffers
    gau_l = nc.dram_tensor("gau_l", [3, ML], F32)
    gau_g = nc.dram_tensor("gau_g", [ND, 3, ML], F32, addr_space="Shared")
    tau_l = nc.dram_tensor("tau_l", [NQ, 1], F32)
    tau_g = nc.dram_tensor("tau_g", [HW, 1], F32, addr_space="Shared")
    scr = [nc.dram_tensor(f"scr{i}", [HW], F32) for i in range(3)]
    # readout rows interleaved in groups of 65 per core: rows 65d..65d+63 are
    # value rows 64d..64d+63, row 65d+64 is a copy of the local norm row, so a
    # single ReduceScatter delivers each core its value slice + global norm.
    ro_l = nc.dram_tensor("ro_l", [(CV // ND + 1) * ND, HW], F32)

    groups = [list(range(ND))]

    from contextlib import ExitStack
    with TileContext(nc) as tc, ExitStack() as es:
        try:
            POOL_E = mybir.EngineType.Pool
            cpool = es.enter_context(tc.tile_pool(name="consts", bufs=1))
            def cload(ap, name):
                return cpool.tile_from(ap, force_copy=True, name=name,
                                       forced_dma_engine=POOL_E)
            msb = cload(msb_d[:], "msb_t")
            q2c = cload(q2c_d[:], "q2c_t")
            mba = cload(mba_d[:], "mba_t")
            q2o = cload(q2o_d[:], "q2o_t")
            a8t = cload(a8t_d[:], "a8t_t")
            iota16 = cload(iota16_c[:], "iota16_t")
            ones_cb = cload(ones_128x1_c[:], "ones_cb_t")
            thr56 = cload(thr56_c[:], "thr56_t")
            b20 = cload(b20_c[:], "b20_t")

            spool = es.enter_context(tc.tile_pool(name="smalls", bufs=1))
            ycg_t = spool.tile([128, NCH], F32)
            xcg_t = spool.tile([128, NCH], F32)
            alp_t = spool.tile([128, NCH], F32)

            def part_to_row(scratch, row_ap, tile_ap):
                nc.sync.dma_start(
                    out=scratch[:].rearrange("(m q) -> q m", q=128), in_=tile_ap)
                nc.sync.dma_start(out=row_ap, in_=scratch[:])

            PH = int(os.environ.get("KPHASE", "99"))

            # ---------------- phase A: argmax per memory row ----------------
            with tc.tile_pool(name="psA", bufs=2, space="PSUM") as psA, \
                 tc.tile_pool(name="wkA", bufs=3) as wkA:
                for m in range(NCH):
                    ps = psA.tile([128, NB, 512], F32)
                    for j in range(NB):
                        nc.tensor.matmul(
                            ps[:, j, :NW],
                            lhsT=msb[0:69, m * 128:(m + 1) * 128].bitcast(F32R),
                            rhs=q2c[0:69, j * NW:(j + 1) * NW].bitcast(F32R),
                            start=True, stop=True)
                    u = wkA.tile([128, HW], F32, tag="u")
                    nc.scalar.activation(
                        u.rearrange("p (j n) -> p j n", n=NW), ps[:, :, :NW],
                        ACT.Copy)
                    m8 = wkA.tile([128, 8], F32, tag="m8")
                    i8 = wkA.tile([128, 8], U32, tag="i8")
                    nc.vector.max(m8, u)
                    nc.vector.max_index(i8, m8, u)
                    idxf = wkA.tile([128, 1], F32, tag="idxf")
                    nc.vector.tensor_copy(idxf, i8[:, 0:1])
                    xm = wkA.tile([128, 1], F32, tag="xm")
                    ym = wkA.tile([128, 1], F32, tag="ym")
                    jnk = wkA.tile([128, H - 1], F32, tag="jnk")
                    # y = #{k in 1..31 : k*W <= idx} = idx // W
                    nc.vector.tensor_scalar(jnk, thr56, idxf, None, op0=ALU.is_le,
                                            op1=ALU.add, accum_out=ym)
                    # x = idx - W*y
                    nc.vector.scalar_tensor_tensor(xm, ym, -float(W), idxf,
                                                   op0=ALU.mult, op1=ALU.add)
                    nc.scalar.activation(ycg_t[:, m:m + 1], ym, ACT.Copy,
                                         scale=CG)
                    nc.scalar.activation(xcg_t[:, m:m + 1], xm, ACT.Copy,
                                         scale=CG)
                    # alpha = a8 + (y^2 + x^2)/GD = a8 + ((y*cg)^2+(x*cg)^2)/2
                    ysq = wkA.tile([128, 1], F32, tag="ysq")
                    nc.vector.tensor_mul(ysq, ycg_t[:, m:m + 1], ycg_t[:, m:m + 1])
                    xsq = wkA.tile([128, 1], F32, tag="xsq")
                    nc.vector.tensor_mul(xsq, xcg_t[:, m:m + 1], xcg_t[:, m:m + 1])
                    ssum = wkA.tile([128, 1], F32, tag="ssum")
                    nc.vector.tensor_add(ssum, ysq, xsq)
                    hsum = wkA.tile([128, 1], F32, tag="hsum")
                    nc.scalar.activation(hsum, ssum, ACT.Copy, scale=0.5)
                    nc.vector.tensor_add(alp_t[:, m:m + 1], hsum, a8t[:, m:m + 1])
                    # stream this chunk's gaussian channels into msb rows
                    # 64..66 for phase C
                    for row, srct in ((64, ycg_t), (65, xcg_t), (66, alp_t)):
                        nc.sync.dma_start(
                            out=msb[row:row + 1, m * 128:(m + 1) * 128],
                            in_=srct[:, m:m + 1])

            # ship the gaussian rows for all memory rows to every core
            part_to_row(scr[0], gau_l[0:1, :], ycg_t[:])
            part_to_row(scr[1], gau_l[1:2, :], xcg_t[:])
            part_to_row(scr[2], gau_l[2:3, :], alp_t[:])
            if PH < 2:
                raise _Trunc()
            nc.gpsimd.collective_compute(
                "AllGather", ALU.bypass, replica_groups=groups,
                ins=[gau_l[:]], outs=[gau_g[:]])
            for r in range(3):
                nc.sync.dma_start(
                    out=mba[64 + r:65 + r, :].rearrange("a (d m) -> a d m",
                                                        d=ND),
                    in_=gau_g[:, r:r + 1, :].rearrange("d a m -> a d m"))

            if PH < 3:
                raise _Trunc()
            # -------- phase B: query-sharded selection over all memory ------
            with tc.tile_pool(name="sSpool", bufs=1) as sSpool, \
                 tc.tile_pool(name="wkB", bufs=1) as wkB, \
                 tc.tile_pool(name="wkT", bufs=2) as wkT, \
                 tc.tile_pool(name="psB", bufs=1, space="PSUM") as psB:
                sS = sSpool.tile([128, 2, THW], F32)
                seg = sSpool.tile([128, 2, NSEG], F32)
                for ci, (p0, pc) in enumerate(((0, 128), (128, 96))):
                    for j in range(NJ):
                        ps = psB.tile([128, 512], F32, tag=f"b{j % 4}")
                        nc.tensor.matmul(
                            ps[0:pc, :NW],
                            lhsT=q2o[:, p0:p0 + pc].bitcast(F32R),
                            rhs=mba[:, j * NW:(j + 1) * NW].bitcast(F32R),
                            start=True, stop=True)
                        nc.scalar.activation(
                            sS[0:pc, ci, j * NW:(j + 1) * NW], ps[0:pc, :NW],
                            ACT.Copy)
                    # segment-16 maxima via 4-round pairwise max tournament on
                    # Pool, batched over groups of 4 column blocks
                    for g in range(NJ // 4):
                        cur = sS[0:pc, ci, g * 4 * NW:(g + 1) * 4 * NW] \
                            .rearrange("p (b n) -> p b n", b=4)
                        w = NW
                        for r in range(4):
                            w //= 2
                            halves = cur.rearrange("p b (s k) -> p b s k", k=2)
                            if r < 3:
                                nxtf = wkB.tile([128, 4, w], F32, tag=f"t{r}", name=f"tt{r}")
                                nxt = nxtf[0:pc]
                            else:
                                nxt = seg[0:pc, ci,
                                          g * 112:(g + 1) * 112] \
                                    .rearrange("p (b s) -> p b s", b=4)
                            nc.gpsimd.scalar_tensor_tensor(
                                nxt, halves[:, :, :, 0], 1.0,
                                halves[:, :, :, 1], op0=ALU.mult, op1=ALU.max)
                            cur = nxt

                if PH < 4:
                    raise _Trunc()

                # ---- selection stages, explicitly interleaved so both
                # chunks' serial chains (rank51 -> v -> z -> max8 -> count)
                # overlap across DVE / Pool / ACT in-order queues ----
                CHK = ((0, 0, 128), (1, 128, 96))
                HWF = THW // 2
                t_cs, mn8s, css = {}, {}, {}

                def r51(ci, p0, pc):
                    ext = seg[0:pc, ci, :]
                    m8f = wkT.tile([128, 8], F32, tag=f"m8{ci}", name="m8")
                    m8 = m8f[0:pc]
                    for r in range(7):
                        nc.vector.max(m8, ext)
                        if r < 6:
                            nc.vector.match_replace(ext, m8, ext, NEG)
                    t_cf = wkT.tile([128, 1], F32, tag=f"t{ci}", name="tc")
                    t_cs[ci] = t_cf[0:pc]
                    nc.vector.tensor_copy(t_cs[ci], m8[:, 2:3])

                def vz(ci, p0, pc, h, eng):
                    # v = t - s ; z = min(v, -BIG*v), in place over the
                    # score half
                    sl = slice(h * HWF, (h + 1) * HWF)
                    Sh = sS[0:pc, ci, sl]
                    eng.tensor_scalar(Sh, Sh, t_cs[ci], -1.0,
                                      op0=ALU.subtract, op1=ALU.mult)
                    eng.scalar_tensor_tensor(
                        Sh, Sh, -BIG, Sh, op0=ALU.mult, op1=ALU.min)

                def m8h(ci, p0, pc, h):
                    # 8 smallest survivors of this half (as t-s, descending)
                    if ci not in mn8s:
                        cdf = wkT.tile([128, 16], F32, tag=f"cd{ci}", name="cd")
                        mn8s[ci] = cdf[0:pc]
                    sl = slice(h * HWF, (h + 1) * HWF)
                    nc.vector.max(mn8s[ci][:, h * 8:(h + 1) * 8],
                                  sS[0:pc, ci, sl])

                def cnt(ci, p0, pc, h):
                    # count survivors of this half on ACT:
                    # sign(z + 1e20) accumulate
                    sl = slice(h * HWF, (h + 1) * HWF)
                    Sh = sS[0:pc, ci, sl]
                    if ci not in css:
                        csf = wkT.tile([128, 2], F32, tag=f"cs{ci}", name="cs")
                        css[ci] = csf[0:pc]
                    nc.scalar.activation(Sh, Sh, ACT.Sign, bias=b20[0:pc],
                                         accum_out=css[ci][:, h:h + 1])

                def tau_fin(ci, p0, pc):
                    # tau = t - (asc8[e] + asc8[e-1])/2 with e = count - 50,
                    # count = (14336 + cs0 + cs1)/2
                    mn8 = wkT.tile([128, 8], F32, tag=f"mn{ci}", name="mn8")[0:pc]
                    nc.vector.max(mn8, mn8s[ci])
                    em05 = wkT.tile([128, 1], F32, tag=f"e{ci}", name="em")[0:pc]
                    nc.vector.tensor_reduce(em05, css[ci], axis=AX.X, op=ALU.add)
                    nc.vector.tensor_scalar(em05, em05, 0.5,
                                            float(THW) / 2 - 50.5,
                                            op0=ALU.mult, op1=ALU.add)
                    d8 = wkT.tile([128, 8], F32, tag=f"d8{ci}", name="d8")[0:pc]
                    nc.vector.tensor_scalar(d8, iota16[0:pc, 0:8], em05, 0.0,
                                            op0=ALU.subtract, op1=ALU.abs_max)
                    mk2 = wkT.tile([128, 8], F32, tag=f"mk{ci}", name="mk2")[0:pc]
                    nc.vector.tensor_scalar(mk2, d8, 0.6, None, op0=ALU.is_le)
                    junk8 = wkT.tile([128, 8], F32, tag=f"j8{ci}", name="j8")[0:pc]
                    msum = wkT.tile([128, 1], F32, tag=f"ms{ci}", name="ms")[0:pc]
                    nc.vector.scalar_tensor_tensor(
                        junk8, mn8, 1.0, mk2, op0=ALU.mult, op1=ALU.mult,
                        accum_out=msum)
                    tau_c = wkT.tile([128, 1], F32, tag=f"tv{ci}", name="tv")[0:pc]
                    nc.vector.scalar_tensor_tensor(
                        tau_c, msum, -0.5, t_cs[ci], op0=ALU.mult, op1=ALU.add)
                    nc.sync.dma_start(out=tau_l[p0:p0 + pc, :], in_=tau_c)

                r51(0, 0, 128)
                vz(0, 0, 128, 0, nc.vector)
                vz(0, 0, 128, 1, nc.gpsimd)
                m8h(0, 0, 128, 0)
                cnt(0, 0, 128, 0)
                r51(1, 128, 96)
                vz(1, 128, 96, 0, nc.vector)
                vz(1, 128, 96, 1, nc.gpsimd)
                m8h(1, 128, 96, 0)
                cnt(1, 128, 96, 0)
                m8h(0, 0, 128, 1)
                cnt(0, 0, 128, 1)
                m8h(1, 128, 96, 1)
                cnt(1, 128, 96, 1)
                tau_fin(0, 0, 128)
                tau_fin(1, 128, 96)

            if PH < 5:
                raise _Trunc()
            nc.gpsimd.collective_compute(
                "AllGather", ALU.bypass, replica_groups=groups,
                ins=[tau_l[:]], outs=[tau_g[:]])
            # tau (absolute) becomes q2c channel 67: psC = s - tau
            nc.sync.dma_start(out=q2c[67:68, :],
                              in_=tau_g[:].rearrange("q s -> s q"))

            if PH < 6:
                raise _Trunc()
            # ---------------- phase C: weights + readout ----------------
            with tc.tile_pool(name="Wpool", bufs=1) as Wpool, \
                 tc.tile_pool(name="mvp", bufs=1) as mvpool:
                Wt = Wpool.tile([128, NCH, ML], BF16)
                mvt = mvpool.tile_from(mvt_d[:], force_copy=True,
                                       forced_dma_engine=POOL_E)
                mvt3 = mvt.rearrange("p (k c) -> p k c", c=CV)
                with tc.tile_pool(name="psC", bufs=2, space="PSUM") as psC, \
                     tc.tile_pool(name="wkF", bufs=3) as wkF:
                    for k in range(NCH):
                        ps = psC.tile([128, NB, 512], F32)
                        for j in range(NB):
                            nc.tensor.matmul(
                                ps[:, j, :NW],
                                lhsT=msb[0:68, k * 128:(k + 1) * 128]
                                .bitcast(F32R),
                                rhs=q2c[0:68, j * NW:(j + 1) * NW]
                                .bitcast(F32R),
                                start=True, stop=True)
                        # psC = s - tau: z = min(ps*BIG, ps) maps rejected
                        # entries (ps<0) to -huge so exp(z) = masked weight;
                        # DVE and Pool each premask half the chunk
                        z = wkF.tile([128, NB, NW], F32, tag="z")
                        nc.vector.scalar_tensor_tensor(
                            z[:, 0:2], ps[:, 0:2, :NW], BIG, ps[:, 0:2, :NW],
                            op0=ALU.mult, op1=ALU.min)
                        nc.gpsimd.scalar_tensor_tensor(
                            z[:, 2:4], ps[:, 2:4, :NW], BIG, ps[:, 2:4, :NW],
                            op0=ALU.mult, op1=ALU.min)
                        nc.scalar.activation(
                            Wt[:, k, :].rearrange("p (j n) -> p j n", n=NW),
                            z, ACT.Exp)

                if PH < 7:
                    raise _Trunc()
                # Readout in two k-halves with SBUF partial accumulation so
                # the first half's matmuls overlap phase C's tail.
                KH = NCH // 2
                with tc.tile_pool(name="wkO", bufs=2) as wkO, \
                     tc.tile_pool(name="accp", bufs=1) as accp:
                    pacc = accp.tile([128, MCV, NB, NW], F32)
                    with tc.tile_pool(name="psO", bufs=2, space="PSUM") as psO:
                        for half in range(2):
                            k0 = half * KH
                            for mc in range(MCV):
                                po = psO.tile([128, NB, 512], F32, tag="po")
                                for k in range(k0, k0 + KH):
                                    for j in range(NB):
                                        nc.tensor.matmul(
                                            po[:, j, :NW],
                                            lhsT=mvt3[:, k,
                                                      mc * 128:(mc + 1) * 128],
                                            rhs=Wt[:, k, j * NW:(j + 1) * NW],
                                            start=(k == k0),
                                            stop=(k == k0 + KH - 1))
                                if half == 0:
                                    nc.scalar.activation(
                                        pacc[:, mc], po[:, :, :NW], ACT.Copy)
                                else:
                                    ob = wkO.tile([128, NB, NW], F32, tag="ob")
                                    nc.vector.tensor_add(
                                        ob, pacc[:, mc], po[:, :, :NW])
                                    # value rows v=64d+i land at ro row 65d+i
                                    for h in range(2):
                                        d = 2 * mc + h
                                        nc.sync.dma_start(
                                            out=ro_l[d * 65:d * 65 + 64, :]
                                            .rearrange("r (j n) -> r j n",
                                                       n=NW),
                                            in_=ob[h * 64:(h + 1) * 64])
                    with tc.tile_pool(name="psN", bufs=1, space="PSUM") as psN:
                        pn = psN.tile([1, NB, 512], F32)
                        for k in range(NCH):
                            for j in range(NB):
                                nc.tensor.matmul(
                                    pn[:, j, :NW], lhsT=ones_cb,
                                    rhs=Wt[:, k, j * NW:(j + 1) * NW],
                                    start=(k == 0), stop=(k == NCH - 1))
                        nb_ = wkO.tile([1, NB, NW], F32, tag="nb")
                        nc.scalar.activation(nb_, pn[:, :, :NW], ACT.Copy)
                        for d in range(ND):
                            nc.sync.dma_start(
                                out=ro_l[d * 65 + 64:d * 65 + 65, :]
                                .rearrange("r (j n) -> r j n", n=NW),
                                in_=nb_)

            if PH < 8:
                raise _Trunc()
            # one ReduceScatter delivers 64 summed value rows + the summed
            # norm row to each core; the host does the division.
            nc.gpsimd.collective_compute(
                "ReduceScatter", ALU.add, replica_groups=groups,
                ins=[ro_l[:]], outs=[out_d[:]])

        except _Trunc:
            pass
    if not nc.is_finalized():
        nc.finalize()
    return nc


def _host_inputs(mk, qk, mv):
    mkf = np.asarray(mk, np.float32).reshape(CK, THW)
    qkf = np.asarray(qk, np.float32).reshape(CK, HW)
    mvf = np.asarray(mv, np.float32).reshape(CV, THW)
    c = (qkf * qkf).sum(0)
    a = (mkf * mkf).sum(0)
    yv = (np.arange(HW, dtype=np.float32) // W)
    xv = (np.arange(HW, dtype=np.float32) % W)

    q2c = np.zeros((69, HW), np.float32)
    q2c[0:64] = qkf
    q2c[64] = yv * CG
    q2c[65] = xv * CG
    q2c[66] = -1.0
    q2c[67] = 0.0
    q2c[68] = c / 8.0

    mba = np.zeros((67, THW), np.float32)
    mba[0:64] = mkf / 4.0

    in_maps = []
    for d in range(ND):
        sl = slice(d * ML, (d + 1) * ML)
        qsl = slice(d * NQ, (d + 1) * NQ)
        msb = np.zeros((69, ML), np.float32)
        msb[0:64] = mkf[:, sl] / 4.0
        msb[67] = -1.0
        msb[68] = -1.0
        q2o = np.zeros((67, NQ), np.float32)
        q2o[0:64] = qkf[:, qsl]
        q2o[64] = (yv * CG)[qsl]
        q2o[65] = (xv * CG)[qsl]
        q2o[66] = -1.0
        a8t = np.ascontiguousarray(
            (a[sl] / 8.0).reshape(NCH, 128).T.astype(np.float32))
        mvt = np.ascontiguousarray(
            mvf[:, sl].T.reshape(NCH, 128, CV).transpose(1, 0, 2)
            .reshape(128, NCH * CV)).astype(ml_dtypes.bfloat16)
        in_maps.append({
            "msb": msb, "q2c": q2c, "mba": mba, "q2o": q2o,
            "a8t": a8t, "mvt": mvt,
        })
    return in_maps


_NC_CACHE = {}


def _get_nc():
    if "nc" not in _NC_CACHE:
        _NC_CACHE["nc"] = _build()
    return _NC_CACHE["nc"]


def assemble(per_core_outs):
    """Each core returns [65, HW]: 64 summed value rows + the summed norm
    row. Normalize host-side and concatenate the 8 slices."""
    parts = []
    for o in per_core_outs:
        o = np.asarray(o, np.float32)
        parts.append(o[0:CV // ND] / o[CV // ND:CV // ND + 1])
    return np.concatenate(parts, axis=0).reshape(1, CV, H, W)


def kernel(mk, qk, mv):
    from concourse.bass_utils import run_bass_kernel_spmd
    in_maps = _host_inputs(mk, qk, mv)
    nc = _get_nc()
    res = run_bass_kernel_spmd(nc, in_maps, core_ids=list(range(ND)))
    return assemble([res.results[d]["out"] for d in range(ND)])


# revision 46
# speedup vs baseline: 1.1803x; 1.0220x over previous
"""Trainium2 Bass kernel for EvalMemoryReader (retrieval_knn).

Distributed plan (8 NeuronCores):
  A. memory-sharded argmax: fp32r matmul (own 1792 memory rows x all 1792
     queries) -> per-row argmax via DVE max8+find-index -> gaussian center
     (ym, xm) and alpha per memory row.  One AllGather ships the three
     gaussian rows for all 14336 memory rows to every core.
  B. query-sharded selection: each core computes scores s(m, q) for its own
     224 queries over ALL 14336 memory rows (fp32r matmul, 67 channels
     folding the gaussian), takes segment-16 maxima via a Pool max
     tournament, rank-51 of the 896 segmaxes via 7x(max8+match_replace)
     -> threshold t, then an in-place 4-pass sweep over the score row
     (v = t-s, z = min(v, -BIG*v), max8 -> 8 smallest survivors, count)
     -> exact v50/v51 midpoint tau per query.  AllGather tau (tiny).
  C. memory-sharded weights: fp32r matmul with tau folded in as a channel
     (psum = s - tau), premask z = min(ps*BIG, ps), exp -> bf16 weights;
     bf16 readout matmul in two k-halves + a norm row; one ReduceScatter
     with 65-row interleaving delivers summed values + norm; host divides.

kernel() takes FULL inputs, shards host-side, runs SPMD on cores 0-7.
"""

import math
import os

import ml_dtypes
import numpy as np

import concourse.bass as bass
import concourse.bacc as bacc
import concourse.mybir as mybir
from concourse.tile import TileContext

ND = 8
CK, CV, T, H, W = 64, 512, 8, 32, 56
HW = H * W              # 1792 queries
THW = T * HW            # 14336 memory locations
ML = THW // ND          # 1792 memory rows per core
NCH = HW // 128         # 14 chunks of 128
NB = 4                  # 448-wide free-dim chunks per 1792
NW = HW // NB           # 448
SEG = 16
NSEG = THW // SEG       # 896 segments per query (global)
NQ = HW // ND           # 224 queries per core
NJ = THW // NW          # 32 column blocks of 448 in the selection matmul
GD = 2.0 * 5.6 * 5.6    # 62.72
CG = math.sqrt(2.0 / GD)
BIG = 1.0e30
NEG = -1.0e30
MCV = CV // 128         # 4 output chunks

F32 = mybir.dt.float32
F32R = mybir.dt.float32r
BF16 = mybir.dt.bfloat16
U32 = mybir.dt.uint32
ALU = mybir.AluOpType
ACT = mybir.ActivationFunctionType
AX = mybir.AxisListType


class _Trunc(Exception):
    pass


def _build():
    nc = bacc.Bacc(num_devices=ND)

    # msb rows: 0-63 own mk/4, 64-66 gaussian rows (runtime), 67 = -1 (tau
    # channel for phase C), 68 = -1 (|q|^2 channel for phase A)
    msb_d = nc.dram_tensor("msb", [69, ML], F32, kind="ExternalInput")
    # q2c rows: 0-63 qk, 64 yv*cg, 65 xv*cg, 66 = -1, 67 = tau (runtime),
    # 68 = |q|^2/8
    q2c_d = nc.dram_tensor("q2c", [69, HW], F32, kind="ExternalInput")
    # selection operands: full-memory channels + own-query columns
    mba_d = nc.dram_tensor("mba", [67, THW], F32, kind="ExternalInput")
    q2o_d = nc.dram_tensor("q2o", [67, NQ], F32, kind="ExternalInput")
    a8t_d = nc.dram_tensor("a8t", [128, NCH], F32, kind="ExternalInput")
    mvt_d = nc.dram_tensor("mvt", [128, NCH * CV], BF16, kind="ExternalInput")
    out_d = nc.dram_tensor("out", [CV // ND + 1, HW], F32, kind="ExternalOutput")

    iota16_c = nc.inline_tensor(
        np.broadcast_to(np.arange(16, dtype=np.float32), (128, 16)).copy(),
        name="iota16")
    ones_128x1_c = nc.inline_tensor(
        np.ones((128, 1), np.float32).astype(ml_dtypes.bfloat16), name="o128x1")
    thr56_c = nc.inline_tensor(
        np.broadcast_to(np.arange(1, H, dtype=np.float32) * W, (128, H - 1))
        .copy(), name="thr56")
    b20_c = nc.inline_tensor(
        np.full((128, 1), 1.0e20, np.float32), name="b20")

    # collective bounce buffers
    gau_l = nc.dram_tensor("gau_l", [3, ML], F32)
    gau_g = nc.dram_tensor("gau_g", [ND, 3, ML], F32, addr_space="Shared")
    tau_l = nc.dram_tensor("tau_l", [NQ, 1], F32)
    tau_g = nc.dram_tensor("tau_g", [HW, 1], F32, addr_space="Shared")
    scr = [nc.dram_tensor(f"scr{i}", [HW], F32) for i in range(3)]
    # readout rows interleaved in groups of 65 per core: rows 65d..65d+63 are
    # value rows 64d..64d+63, row 65d+64 is a copy of the local norm row, so a
    # single ReduceScatter delivers each core its value slice + global norm.
    ro_l = nc.dram_tensor("ro_l", [(CV // ND + 1) * ND, HW], F32)

    groups = [list(range(ND))]

    from contextlib import ExitStack
    with TileContext(nc) as tc, ExitStack() as es:
        try:
            POOL_E = mybir.EngineType.Pool
            cpool = es.enter_context(tc.tile_pool(name="consts", bufs=1))
            def cload(ap, name):
                return cpool.tile_from(ap, force_copy=True, name=name,
                                       forced_dma_engine=POOL_E)
            msb = cload(msb_d[:], "msb_t")
            q2c = cload(q2c_d[:], "q2c_t")
            mba = cload(mba_d[:], "mba_t")
            q2o = cload(q2o_d[:], "q2o_t")
            a8t = cload(a8t_d[:], "a8t_t")
            iota16 = cload(iota16_c[:], "iota16_t")
            ones_cb = cload(ones_128x1_c[:], "ones_cb_t")
            thr56 = cload(thr56_c[:], "thr56_t")
            b20 = cload(b20_c[:], "b20_t")

            spool = es.enter_context(tc.tile_pool(name="smalls", bufs=1))
            ycg_t = spool.tile([128, NCH], F32)
            xcg_t = spool.tile([128, NCH], F32)
            alp_t = spool.tile([128, NCH], F32)

            def part_to_row(scratch, row_ap, tile_ap):
                nc.sync.dma_start(
                    out=scratch[:].rearrange("(m q) -> q m", q=128), in_=tile_ap)
                nc.sync.dma_start(out=row_ap, in_=scratch[:])

            PH = int(os.environ.get("KPHASE", "99"))

            # ---------------- phase A: argmax per memory row ----------------
            with tc.tile_pool(name="psA", bufs=2, space="PSUM") as psA, \
                 tc.tile_pool(name="wkA", bufs=3) as wkA:
                for m in range(NCH):
                    ps = psA.tile([128, NB, 512], F32)
                    for j in range(NB):
                        nc.tensor.matmul(
                            ps[:, j, :NW],
                            lhsT=msb[0:69, m * 128:(m + 1) * 128].bitcast(F32R),
                            rhs=q2c[0:69, j * NW:(j + 1) * NW].bitcast(F32R),
                            start=True, stop=True)
                    u = wkA.tile([128, HW], F32, tag="u")
                    nc.scalar.activation(
                        u.rearrange("p (j n) -> p j n", n=NW), ps[:, :, :NW],
                        ACT.Copy)
                    m8 = wkA.tile([128, 8], F32, tag="m8")
                    i8 = wkA.tile([128, 8], U32, tag="i8")
                    nc.vector.max(m8, u)
                    nc.vector.max_index(i8, m8, u)
                    idxf = wkA.tile([128, 1], F32, tag="idxf")
                    nc.vector.tensor_copy(idxf, i8[:, 0:1])
                    xm = wkA.tile([128, 1], F32, tag="xm")
                    ym = wkA.tile([128, 1], F32, tag="ym")
                    jnk = wkA.tile([128, H - 1], F32, tag="jnk")
                    # y = #{k in 1..31 : k*W <= idx} = idx // W
                    nc.vector.tensor_scalar(jnk, thr56, idxf, None, op0=ALU.is_le,
                                            op1=ALU.add, accum_out=ym)
                    # x = idx - W*y
                    nc.vector.scalar_tensor_tensor(xm, ym, -float(W), idxf,
                                                   op0=ALU.mult, op1=ALU.add)
                    nc.scalar.activation(ycg_t[:, m:m + 1], ym, ACT.Copy,
                                         scale=CG)
                    nc.scalar.activation(xcg_t[:, m:m + 1], xm, ACT.Copy,
                                         scale=CG)
                    # alpha = a8 + (y^2 + x^2)/GD = a8 + ((y*cg)^2+(x*cg)^2)/2
                    ysq = wkA.tile([128, 1], F32, tag="ysq")
                    nc.vector.tensor_mul(ysq, ycg_t[:, m:m + 1], ycg_t[:, m:m + 1])
                    xsq = wkA.tile([128, 1], F32, tag="xsq")
                    nc.vector.tensor_mul(xsq, xcg_t[:, m:m + 1], xcg_t[:, m:m + 1])
                    ssum = wkA.tile([128, 1], F32, tag="ssum")
                    nc.vector.tensor_add(ssum, ysq, xsq)
                    hsum = wkA.tile([128, 1], F32, tag="hsum")
                    nc.scalar.activation(hsum, ssum, ACT.Copy, scale=0.5)
                    nc.vector.tensor_add(alp_t[:, m:m + 1], hsum, a8t[:, m:m + 1])
                    # stream this chunk's gaussian channels into msb rows
                    # 64..66 for phase C
                    for row, srct in ((64, ycg_t), (65, xcg_t), (66, alp_t)):
                        nc.sync.dma_start(
                            out=msb[row:row + 1, m * 128:(m + 1) * 128],
                            in_=srct[:, m:m + 1])

            # ship the gaussian rows for all memory rows to every core
            part_to_row(scr[0], gau_l[0:1, :], ycg_t[:])
            part_to_row(scr[1], gau_l[1:2, :], xcg_t[:])
            part_to_row(scr[2], gau_l[2:3, :], alp_t[:])
            if PH < 2:
                raise _Trunc()
            nc.gpsimd.collective_compute(
                "AllGather", ALU.bypass, replica_groups=groups,
                ins=[gau_l[:]], outs=[gau_g[:]])
            for r in range(3):
                nc.sync.dma_start(
                    out=mba[64 + r:65 + r, :].rearrange("a (d m) -> a d m",
                                                        d=ND),
                    in_=gau_g[:, r:r + 1, :].rearrange("d a m -> a d m"))

            if PH < 3:
                raise _Trunc()
            # -------- phase B: query-sharded selection over all memory ------
            with tc.tile_pool(name="sSpool", bufs=1) as sSpool, \
                 tc.tile_pool(name="wkB", bufs=1) as wkB, \
                 tc.tile_pool(name="wkT", bufs=2) as wkT, \
                 tc.tile_pool(name="psB", bufs=1, space="PSUM") as psB:
                sS = sSpool.tile([128, 2, THW], F32)
                seg = sSpool.tile([128, 2, NSEG], F32)
                for ci, (p0, pc) in enumerate(((0, 128), (128, 96))):
                    for j in range(NJ):
                        ps = psB.tile([128, 512], F32, tag=f"b{j % 4}")
                        nc.tensor.matmul(
                            ps[0:pc, :NW],
                            lhsT=q2o[:, p0:p0 + pc].bitcast(F32R),
                            rhs=mba[:, j * NW:(j + 1) * NW].bitcast(F32R),
                            start=True, stop=True)
                        nc.scalar.activation(
                            sS[0:pc, ci, j * NW:(j + 1) * NW], ps[0:pc, :NW],
                            ACT.Copy)
                        if j % 4 != 3:
                            continue
                        # segment-16 maxima for the group of 4 column blocks
                        # just copied (streams behind the copies): even
                        # groups as one segmented tensor_reduce on DVE, odd
                        # groups as a 4-round pairwise max tournament on Pool
                        g = j // 4
                        gsl = slice(g * 4 * NW, (g + 1) * 4 * NW)
                        if g % 2 == 0:
                            nc.vector.tensor_reduce(
                                seg[0:pc, ci, g * 112:(g + 1) * 112],
                                sS[0:pc, ci, gsl]
                                .rearrange("p (s k) -> p s k", k=SEG),
                                axis=AX.X, op=ALU.max)
                            continue
                        cur = sS[0:pc, ci, gsl] \
                            .rearrange("p (b n) -> p b n", b=4)
                        w = NW
                        for r in range(4):
                            w //= 2
                            halves = cur.rearrange("p b (s k) -> p b s k", k=2)
                            if r < 3:
                                nxtf = wkB.tile([128, 4, w], F32, tag=f"t{r}", name=f"tt{r}")
                                nxt = nxtf[0:pc]
                            else:
                                nxt = seg[0:pc, ci,
                                          g * 112:(g + 1) * 112] \
                                    .rearrange("p (b s) -> p b s", b=4)
                            nc.gpsimd.scalar_tensor_tensor(
                                nxt, halves[:, :, :, 0], 1.0,
                                halves[:, :, :, 1], op0=ALU.mult, op1=ALU.max)
                            cur = nxt

                if PH < 4:
                    raise _Trunc()

                # ---- selection stages, explicitly interleaved so both
                # chunks' serial chains (rank51 -> v -> z -> max8 -> count)
                # overlap across DVE / Pool / ACT in-order queues ----
                CHK = ((0, 0, 128), (1, 128, 96))
                # DVE also carries rank51 + max8, so give it the smaller
                # share of the v/z sweeps and Pool the rest
                HCUT = (0, 7168, THW)
                t_cs, mn8s, css = {}, {}, {}

                def r51(ci, p0, pc):
                    ext = seg[0:pc, ci, :]
                    m8f = wkT.tile([128, 8], F32, tag=f"m8{ci}", name="m8")
                    m8 = m8f[0:pc]
                    for r in range(7):
                        nc.vector.max(m8, ext)
                        if r < 6:
                            nc.vector.match_replace(ext, m8, ext, NEG)
                    t_cf = wkT.tile([128, 1], F32, tag=f"t{ci}", name="tc")
                    t_cs[ci] = t_cf[0:pc]
                    nc.vector.tensor_copy(t_cs[ci], m8[:, 2:3])

                def vz(ci, p0, pc, h, eng):
                    # v = t - s ; z = min(v, -BIG*v), in place over the
                    # score part
                    sl = slice(HCUT[h], HCUT[h + 1])
                    Sh = sS[0:pc, ci, sl]
                    eng.tensor_scalar(Sh, Sh, t_cs[ci], -1.0,
                                      op0=ALU.subtract, op1=ALU.mult)
                    eng.scalar_tensor_tensor(
                        Sh, Sh, -BIG, Sh, op0=ALU.mult, op1=ALU.min)

                def m8h(ci, p0, pc, h):
                    # 8 smallest survivors of this half (as t-s, descending)
                    if ci not in mn8s:
                        cdf = wkT.tile([128, 16], F32, tag=f"cd{ci}", name="cd")
                        mn8s[ci] = cdf[0:pc]
                    sl = slice(HCUT[h], HCUT[h + 1])
                    nc.vector.max(mn8s[ci][:, h * 8:(h + 1) * 8],
                                  sS[0:pc, ci, sl])

                def cnt(ci, p0, pc, h):
                    # count survivors of this half on ACT:
                    # sign(z + 1e20) accumulate
                    sl = slice(HCUT[h], HCUT[h + 1])
                    Sh = sS[0:pc, ci, sl]
                    if ci not in css:
                        csf = wkT.tile([128, 2], F32, tag=f"cs{ci}", name="cs")
                        css[ci] = csf[0:pc]
                    nc.scalar.activation(Sh, Sh, ACT.Sign, bias=b20[0:pc],
                                         accum_out=css[ci][:, h:h + 1])

                def tau_fin(ci, p0, pc):
                    # tau = t - (asc8[e] + asc8[e-1])/2 with e = count - 50,
                    # count = (14336 + cs0 + cs1)/2
                    mn8 = wkT.tile([128, 8], F32, tag=f"mn{ci}", name="mn8")[0:pc]
                    nc.vector.max(mn8, mn8s[ci])
                    em05 = wkT.tile([128, 1], F32, tag=f"e{ci}", name="em")[0:pc]
                    nc.vector.tensor_reduce(em05, css[ci], axis=AX.X, op=ALU.add)
                    nc.vector.tensor_scalar(em05, em05, 0.5,
                                            float(THW) / 2 - 50.5,
                                            op0=ALU.mult, op1=ALU.add)
                    d8 = wkT.tile([128, 8], F32, tag=f"d8{ci}", name="d8")[0:pc]
                    nc.vector.tensor_scalar(d8, iota16[0:pc, 0:8], em05, 0.0,
                                            op0=ALU.subtract, op1=ALU.abs_max)
                    mk2 = wkT.tile([128, 8], F32, tag=f"mk{ci}", name="mk2")[0:pc]
                    nc.vector.tensor_scalar(mk2, d8, 0.6, None, op0=ALU.is_le)
                    junk8 = wkT.tile([128, 8], F32, tag=f"j8{ci}", name="j8")[0:pc]
                    msum = wkT.tile([128, 1], F32, tag=f"ms{ci}", name="ms")[0:pc]
                    nc.vector.scalar_tensor_tensor(
                        junk8, mn8, 1.0, mk2, op0=ALU.mult, op1=ALU.mult,
                        accum_out=msum)
                    tau_c = wkT.tile([128, 1], F32, tag=f"tv{ci}", name="tv")[0:pc]
                    nc.vector.scalar_tensor_tensor(
                        tau_c, msum, -0.5, t_cs[ci], op0=ALU.mult, op1=ALU.add)
                    nc.sync.dma_start(out=tau_l[p0:p0 + pc, :], in_=tau_c)

                r51(0, 0, 128)
                vz(0, 0, 128, 0, nc.vector)
                vz(0, 0, 128, 1, nc.gpsimd)
                m8h(0, 0, 128, 0)
                cnt(0, 0, 128, 0)
                r51(1, 128, 96)
                vz(1, 128, 96, 0, nc.vector)
                vz(1, 128, 96, 1, nc.gpsimd)
                m8h(1, 128, 96, 0)
                cnt(1, 128, 96, 0)
                m8h(0, 0, 128, 1)
                cnt(0, 0, 128, 1)
                m8h(1, 128, 96, 1)
                cnt(1, 128, 96, 1)
                tau_fin(0, 0, 128)
                tau_fin(1, 128, 96)

            if PH < 5:
                raise _Trunc()
            nc.gpsimd.collective_compute(
                "AllGather", ALU.bypass, replica_groups=groups,
                ins=[tau_l[:]], outs=[tau_g[:]])
            # tau (absolute) becomes q2c channel 67: psC = s - tau
            nc.sync.dma_start(out=q2c[67:68, :],
                              in_=tau_g[:].rearrange("q s -> s q"))

            if PH < 6:
                raise _Trunc()
            # ---------------- phase C: weights + readout ----------------
            with tc.tile_pool(name="Wpool", bufs=1) as Wpool, \
                 tc.tile_pool(name="mvp", bufs=1) as mvpool:
                Wt = Wpool.tile([128, NCH, ML], BF16)
                mvt = mvpool.tile_from(mvt_d[:], force_copy=True,
                                       forced_dma_engine=POOL_E)
                mvt3 = mvt.rearrange("p (k c) -> p k c", c=CV)
                with tc.tile_pool(name="psC", bufs=2, space="PSUM") as psC, \
                     tc.tile_pool(name="wkF", bufs=3) as wkF:
                    for k in range(NCH):
                        ps = psC.tile([128, NB, 512], F32)
                        for j in range(NB):
                            nc.tensor.matmul(
                                ps[:, j, :NW],
                                lhsT=msb[0:68, k * 128:(k + 1) * 128]
                                .bitcast(F32R),
                                rhs=q2c[0:68, j * NW:(j + 1) * NW]
                                .bitcast(F32R),
                                start=True, stop=True)
                        # psC = s - tau: z = min(ps*BIG, ps) maps rejected
                        # entries (ps<0) to -huge so exp(z) = masked weight;
                        # DVE and Pool each premask half the chunk
                        z = wkF.tile([128, NB, NW], F32, tag="z")
                        nc.vector.scalar_tensor_tensor(
                            z[:, 0:2], ps[:, 0:2, :NW], BIG, ps[:, 0:2, :NW],
                            op0=ALU.mult, op1=ALU.min)
                        nc.gpsimd.scalar_tensor_tensor(
                            z[:, 2:4], ps[:, 2:4, :NW], BIG, ps[:, 2:4, :NW],
                            op0=ALU.mult, op1=ALU.min)
                        nc.scalar.activation(
                            Wt[:, k, :].rearrange("p (j n) -> p j n", n=NW),
                            z, ACT.Exp)

                if PH < 7:
                    raise _Trunc()
                # Readout in two k-halves with SBUF partial accumulation so
                # the first half's matmuls overlap phase C's tail.
                KH = NCH // 2
                with tc.tile_pool(name="wkO", bufs=2) as wkO, \
                     tc.tile_pool(name="accp", bufs=1) as accp:
                    pacc = accp.tile([128, MCV, NB, NW], F32)
                    with tc.tile_pool(name="psO", bufs=2, space="PSUM") as psO:
                        for half in range(2):
                            k0 = half * KH
                            for mc in range(MCV):
                                po = psO.tile([128, NB, 512], F32, tag="po")
                                for k in range(k0, k0 + KH):
                                    for j in range(NB):
                                        nc.tensor.matmul(
                                            po[:, j, :NW],
                                            lhsT=mvt3[:, k,
                                                      mc * 128:(mc + 1) * 128],
                                            rhs=Wt[:, k, j * NW:(j + 1) * NW],
                                            start=(k == k0),
                                            stop=(k == k0 + KH - 1))
                                if half == 0:
                                    nc.scalar.activation(
                                        pacc[:, mc], po[:, :, :NW], ACT.Copy)
                                else:
                                    ob = wkO.tile([128, NB, NW], F32, tag="ob")
                                    nc.vector.tensor_add(
                                        ob, pacc[:, mc], po[:, :, :NW])
                                    # value rows v=64d+i land at ro row 65d+i
                                    for h in range(2):
                                        d = 2 * mc + h
                                        nc.sync.dma_start(
                                            out=ro_l[d * 65:d * 65 + 64, :]
                                            .rearrange("r (j n) -> r j n",
                                                       n=NW),
                                            in_=ob[h * 64:(h + 1) * 64])
                    with tc.tile_pool(name="psN", bufs=1, space="PSUM") as psN:
                        pn = psN.tile([1, NB, 512], F32)
                        for k in range(NCH):
                            for j in range(NB):
                                nc.tensor.matmul(
                                    pn[:, j, :NW], lhsT=ones_cb,
                                    rhs=Wt[:, k, j * NW:(j + 1) * NW],
                                    start=(k == 0), stop=(k == NCH - 1))
                        nb_ = wkO.tile([1, NB, NW], F32, tag="nb")
                        nc.scalar.activation(nb_, pn[:, :, :NW], ACT.Copy)
                        for d in range(ND):
                            nc.sync.dma_start(
                                out=ro_l[d * 65 + 64:d * 65 + 65, :]
                                .rearrange("r (j n) -> r j n", n=NW),
                                in_=nb_)

            if PH < 8:
                raise _Trunc()
            # one ReduceScatter delivers 64 summed value rows + the summed
            # norm row to each core; the host does the division.
            nc.gpsimd.collective_compute(
                "ReduceScatter", ALU.add, replica_groups=groups,
                ins=[ro_l[:]], outs=[out_d[:]])

        except _Trunc:
            pass
    if not nc.is_finalized():
        nc.finalize()
    return nc


def _host_inputs(mk, qk, mv):
    mkf = np.asarray(mk, np.float32).reshape(CK, THW)
    qkf = np.asarray(qk, np.float32).reshape(CK, HW)
    mvf = np.asarray(mv, np.float32).reshape(CV, THW)
    c = (qkf * qkf).sum(0)
    a = (mkf * mkf).sum(0)
    yv = (np.arange(HW, dtype=np.float32) // W)
    xv = (np.arange(HW, dtype=np.float32) % W)

    q2c = np.zeros((69, HW), np.float32)
    q2c[0:64] = qkf
    q2c[64] = yv * CG
    q2c[65] = xv * CG
    q2c[66] = -1.0
    q2c[67] = 0.0
    q2c[68] = c / 8.0

    mba = np.zeros((67, THW), np.float32)
    mba[0:64] = mkf / 4.0

    in_maps = []
    for d in range(ND):
        sl = slice(d * ML, (d + 1) * ML)
        qsl = slice(d * NQ, (d + 1) * NQ)
        msb = np.zeros((69, ML), np.float32)
        msb[0:64] = mkf[:, sl] / 4.0
        msb[67] = -1.0
        msb[68] = -1.0
        q2o = np.zeros((67, NQ), np.float32)
        q2o[0:64] = qkf[:, qsl]
        q2o[64] = (yv * CG)[qsl]
        q2o[65] = (xv * CG)[qsl]
        q2o[66] = -1.0
        a8t = np.ascontiguousarray(
            (a[sl] / 8.0).reshape(NCH, 128).T.astype(np.float32))
        mvt = np.ascontiguousarray(
            mvf[:, sl].T.reshape(NCH, 128, CV).transpose(1, 0, 2)
            .reshape(128, NCH * CV)).astype(ml_dtypes.bfloat16)
        in_maps.append({
            "msb": msb, "q2c": q2c, "mba": mba, "q2o": q2o,
            "a8t": a8t, "mvt": mvt,
        })
    return in_maps


_NC_CACHE = {}


def _get_nc():
    if "nc" not in _NC_CACHE:
        _NC_CACHE["nc"] = _build()
    return _NC_CACHE["nc"]


def assemble(per_core_outs):
    """Each core returns [65, HW]: 64 summed value rows + the summed norm
    row. Normalize host-side and concatenate the 8 slices."""
    parts = []
    for o in per_core_outs:
        o = np.asarray(o, np.float32)
        parts.append(o[0:CV // ND] / o[CV // ND:CV // ND + 1])
    return np.concatenate(parts, axis=0).reshape(1, CV, H, W)


def kernel(mk, qk, mv):
    from concourse.bass_utils import run_bass_kernel_spmd
    in_maps = _host_inputs(mk, qk, mv)
    nc = _get_nc()
    res = run_bass_kernel_spmd(nc, in_maps, core_ids=list(range(ND)))
    return assemble([res.results[d]["out"] for d in range(ND)])
